# revision 1
# baseline (speedup 1.0000x reference)
"""Trainium2 Bass kernel for nn_G3DCrossAttention (B=2, C=512, L=2048, G=2048, H=8).

Key algebraic structure exploited (exact math, not an approximation of the model):
  exp_p[g,b,:] = exp[b,g]*Wg[:,0] + bg is rank-1 in the channel dim, so
    k[g,b,:] = exp[b,g]*u_k + c_k,   v[g,b,:] = exp[b,g]*u_v + c_v
  with u_k = Wk@Wg, u_v = Wv@Wg, c_v = Wv@bg + bv (all computed on device).
  Scores become scale*(a_i*e_j + d_i) with a = x_seq @ M + a0 (M = Wq.T@(u_k masked
  per head)); the constant-in-j shift d_i cancels in softmax. The attention output
  collapses to
    x_attn = w*u_v + c_v  per head,  w_i = sum_j e_j softmax_j(a_i e_j).
  w = f_b(a) is a smooth scalar function per batch; it is evaluated exactly at 64
  Chebyshev nodes per batch on-device (exp + weighted sums over all G=2048 e_j
  values, both batches in one 128-partition pass), fit with a degree-24 Chebyshev
  series (one matmul with a constant block-diagonal DCT matrix), and evaluated at
  all a values by a Clenshaw recurrence on the vector engine.
  (Validated offline: max |w - w_exact| ~ 5e-6 across all 32768 query/head points;
  |a| <= 4.43 < SCAL = 5, and max |a*e| ~ 15.3 so exp never overflows in fp32.)

Precision: the three big matmuls (FFN1/FFN2/Wo) run in fp16 (11-bit mantissa,
fp32 PSUM accumulate); LN scale/shift outer-products run in f32r. End-to-end
max/absmax error vs the fp32 reference ~ 4e-4 (validated in emulation and HW).

Sharding: data-parallel over L across 8 cores (L/8 = 256 queries each, all heads,
both batches). Each core runs the full FFN/LN/output pipeline for its 512 tokens.
"""

from contextlib import ExitStack

import ml_dtypes
import numpy as np

import concourse.bass as bass
import concourse.tile as tile
from concourse import bacc, mybir
from concourse.bass_utils import run_bass_kernel_spmd

F32 = mybir.dt.float32
F32R = mybir.dt.float32r
FP16 = mybir.dt.float16
AF = mybir.ActivationFunctionType
OP = mybir.AluOpType

B, C, L, G, H = 2, 512, 2048, 2048, 8
D = C // H
NCORES = 8
LC = L // NCORES              # 256 queries per core
T = B * LC                    # 512 tokens per core (tau = b*LC + l)
KC = C // 128                 # 4 partition tiles over C
KH = (4 * C) // 128           # 16 partition tiles over 4C
FP = LC // 8                  # 32: free dim of the packed a/w tiles
SCALE = 1.0 / float(np.sqrt(D))
EPS = 1e-5
SCAL = 5.0                    # Chebyshev half-range in a-units (|a|max ~ 4.43)
KDEG = 24                     # Chebyshev series length
MNODES = 64                   # Chebyshev nodes per batch (2 batches -> 128 parts)

TRACE = False                 # set True to capture an NTFF profile on the next run
TRACE_KW = {}
LAST_RESULTS = None           # BassKernelResults of the most recent run

_CACHE = None


def _consts():
    m = np.arange(MNODES)
    theta = np.pi * (2 * m + 1) / (2 * MNODES)
    xn64 = (SCAL * np.cos(theta)).astype(np.float32)
    xnodes = np.concatenate([xn64, xn64])                 # [128] both batches
    dct1 = np.zeros((MNODES, KDEG), np.float32)
    for k in range(KDEG):
        dct1[:, k] = (2.0 / MNODES) * np.cos(k * theta)
    dct1[:, 0] *= 0.5
    dctbd = np.zeros((2 * MNODES, 2 * KDEG), np.float32)  # block-diag [128, 48]
    dctbd[:MNODES, :KDEG] = dct1
    dctbd[MNODES:, KDEG:] = dct1
    maskc = np.zeros((C, H), np.float32)
    for h in range(H):
        maskc[h * D:(h + 1) * D, h] = 1.0
    return xnodes, dctbd, maskc


def _build():
    nc = bacc.Bacc(debug=False, num_devices=NCORES)

    # ---- external inputs -------------------------------------------------
    seq_sl = nc.dram_tensor("seq_sl", [B, C, LC], F32, kind="ExternalInput")
    expv = nc.dram_tensor("expv", [B, G], F32, kind="ExternalInput")
    wq = nc.dram_tensor("wq", [C, C], F32, kind="ExternalInput")        # Wq as stored
    wkt = nc.dram_tensor("wkt", [C, C], F32, kind="ExternalInput")      # Wk.T
    wvt = nc.dram_tensor("wvt", [C, C], F32, kind="ExternalInput")      # Wv.T
    w1t = nc.dram_tensor("w1t", [C, 4 * C], FP16, kind="ExternalInput")  # W1.T fp16
    w2t = nc.dram_tensor("w2t", [4 * C, C], FP16, kind="ExternalInput")  # W2.T fp16
    wot = nc.dram_tensor("wot", [C, C], FP16, kind="ExternalInput")      # Wo.T fp16
    wg = nc.dram_tensor("wg", [C, 1], F32, kind="ExternalInput")
    bgv = nc.dram_tensor("bgv", [C], F32, kind="ExternalInput")
    bqv = nc.dram_tensor("bqv", [C], F32, kind="ExternalInput")
    bvv = nc.dram_tensor("bvv", [C], F32, kind="ExternalInput")
    b1v = nc.dram_tensor("b1v", [4 * C], F32, kind="ExternalInput")
    b2v = nc.dram_tensor("b2v", [C], F32, kind="ExternalInput")
    bov = nc.dram_tensor("bov", [C], F32, kind="ExternalInput")
    g1v = nc.dram_tensor("g1v", [C], F32, kind="ExternalInput")
    be1 = nc.dram_tensor("be1", [C], F32, kind="ExternalInput")
    g2v = nc.dram_tensor("g2v", [C], F32, kind="ExternalInput")
    be2 = nc.dram_tensor("be2", [C], F32, kind="ExternalInput")

    out_sl = nc.dram_tensor("out_sl", [B, C, LC], F32, kind="ExternalOutput")

    # ---- dram scratch ----------------------------------------------------
    w_dram = nc.dram_tensor("w_scr", [B, H, LC], F32)
    ck_dram = nc.dram_tensor("ck_scr", [B, KDEG], F32)

    # ---- inline constants ------------------------------------------------
    xnodes_np, dct_np, maskc_np = _consts()
    c_xn = nc.inline_tensor(xnodes_np, name="c_xn")
    c_dct = nc.inline_tensor(dct_np, name="c_dct")
    c_mask = nc.inline_tensor(maskc_np, name="c_mask")
    c_onesk_h = nc.inline_tensor(
        np.full(128, 1.0 / C, np.float16), name="c_oneskh")

    with tile.TileContext(nc) as tc, ExitStack() as ctx:
        p_w1 = ctx.enter_context(tc.tile_pool(name="w1", bufs=KC))
        p_w2 = ctx.enter_context(tc.tile_pool(name="w2", bufs=16))
        p_kvh = ctx.enter_context(tc.tile_pool(name="kvh", bufs=8))
        p_wo = ctx.enter_context(tc.tile_pool(name="wo", bufs=KC))
        p_xsz = ctx.enter_context(tc.tile_pool(name="xsz", bufs=4))
        p_act = ctx.enter_context(tc.tile_pool(name="act", bufs=4))
        p_node = ctx.enter_context(tc.tile_pool(name="node", bufs=1))
        p_sm = ctx.enter_context(tc.tile_pool(name="sm", bufs=1))
        p_cl = ctx.enter_context(tc.tile_pool(name="cl", bufs=1))
        ps_mm = ctx.enter_context(tc.tile_pool(name="psmm", bufs=4, space="PSUM"))
        ps_x = ctx.enter_context(tc.tile_pool(name="psx", bufs=1, space="PSUM"))

        # ---- small / stage-A critical loads on the sync queue ------------
        wg_c = [p_sm.tile([128, 2], F32, tag=f"wgbg{kt}", name=f"wgbg_{kt}")
                for kt in range(KC)]
        for kt in range(KC):
            sl = slice(kt * 128, (kt + 1) * 128)
            nc.sync.dma_start(wg_c[kt][:, 0:1], wg[sl, :])
            nc.sync.dma_start(wg_c[kt][:, 1:2], bgv[sl][:, None])
        wkt_t = [p_kvh.tile([128, C], F32, tag="kv", name=f"wkt_{i}")
                 for i in range(KC)]
        wvt_t = [p_kvh.tile([128, C], F32, tag="kv", name=f"wvt_{i}")
                 for i in range(KC)]
        for kt in range(KC):
            nc.sync.dma_start(wkt_t[kt][:], wkt[kt * 128:(kt + 1) * 128, :])
            nc.sync.dma_start(wvt_t[kt][:], wvt[kt * 128:(kt + 1) * 128, :])
        wq_t = [p_w2.tile([128, C], F32, tag="wq", name=f"wq_{i}")
                for i in range(KC)]
        for kt in range(KC):
            nc.sync.dma_start(wq_t[kt][:], wq[kt * 128:(kt + 1) * 128, :])


        dct_sb = p_sm.tile([128, 2 * KDEG], F32, tag="dct")
        nc.sync.dma_start(dct_sb[:], c_dct[:])
        xn_col = p_sm.tile([128, 1], F32, tag="xn")
        nc.sync.dma_start(xn_col[:], c_xn[:])
        mask_t = [p_sm.tile([128, H], F32, tag=f"mask{kt}", name=f"mask_{kt}")
                  for kt in range(KC)]
        for kt in range(KC):
            nc.sync.dma_start(mask_t[kt][:], c_mask[kt * 128:(kt + 1) * 128, :])
        onesk_h = p_sm.tile([128, 1], FP16, tag="oneskh")
        nc.sync.dma_start(onesk_h[:], c_onesk_h[:, None])
        eps_col = p_sm.tile([1, 1], F32, tag="epsc")
        nc.vector.memset(eps_col[:], EPS)

        def col_tiles(src, n, nm, eng=None):
            eng = eng or nc.gpsimd
            ts = [p_sm.tile([128, 1], F32, tag=f"{nm}{i}", name=f"{nm}_{i}")
                  for i in range(n)]
            for i in range(n):
                eng.dma_start(ts[i][:], src[i * 128:(i + 1) * 128][:, None])
            return ts

        bq_c = col_tiles(bqv, KC, "bq", nc.sync)
        bv_c = col_tiles(bvv, KC, "bv", nc.sync)
        bo_c = col_tiles(bov, KC, "bo")
        b1_c = col_tiles(b1v, KH, "b1")
        b2_c = col_tiles(b2v, KC, "b2")
        be1_c = col_tiles(be1, KC, "be1")
        be2_c = col_tiles(be2, KC, "be2")
        g1_row = p_sm.tile([1, C], F32R, tag="g1r")
        nc.sync.dma_start(g1_row[:], g1v[None, :].bitcast(F32R))
        g2_row = p_sm.tile([1, C], F32R, tag="g2r")
        nc.sync.dma_start(g2_row[:], g2v[None, :].bitcast(F32R))

        # x_seq tiles (f32r for the f32r a-matmul): xs[kt][p, tau]
        xs_t = [p_xsz.tile([128, T], F32R, tag="xs", name=f"xs_{i}")
                for i in range(KC)]
        for kt in range(KC):
            src = seq_sl[:, kt * 128:(kt + 1) * 128, :].rearrange("b c l -> c b l")
            nc.sync.dma_start(xs_t[kt][:], src.bitcast(F32R))

        # ---- bulk fp16 weight loads on the (otherwise idle) gpsimd queue -
        w1_t = [p_w1.tile([128, 4 * C], FP16, tag="w1", name=f"w1_{i}")
                for i in range(KC)]
        for kt in range(KC):
            nc.gpsimd.dma_start(w1_t[kt][:], w1t[kt * 128:(kt + 1) * 128, :])
        w2_t = [p_w2.tile([128, C], FP16, tag="w2", name=f"w2_{i}")
                for i in range(KH)]
        for kt in range(KH):
            nc.gpsimd.dma_start(w2_t[kt][:], w2t[kt * 128:(kt + 1) * 128, :])
        wo_t = [p_wo.tile([128, C], FP16, tag="wo", name=f"wo_{i}")
                for i in range(KC)]
        for kt in range(KC):
            nc.gpsimd.dma_start(wo_t[kt][:], wot[kt * 128:(kt + 1) * 128, :])

        # ---- stage A: u_k, u_v, c_v, U, M, a0 ----------------------------
        uk_c, uv_c, cv_c, u_t, m_t = [], [], [], [], []
        for mt in range(KC):
            pk = ps_x.tile([128, 2], F32, tag="small", name=f"pk{mt}")
            for kt in range(KC):
                nc.tensor.matmul(pk[:, 0:1], wkt_t[kt][:, mt * 128:(mt + 1) * 128],
                                 wg_c[kt][:, 0:1], start=(kt == 0), stop=(kt == KC - 1))
            ukc = p_sm.tile([128, 1], F32, tag=f"uk{mt}", name=f"uk_{mt}")
            nc.vector.tensor_copy(ukc[:], pk[:, 0:1])
            uk_c.append(ukc)
            pv = ps_x.tile([128, 2], F32, tag="small", name=f"pv{mt}")
            for kt in range(KC):
                nc.tensor.matmul(pv[:], wvt_t[kt][:, mt * 128:(mt + 1) * 128],
                                 wg_c[kt][:], start=(kt == 0), stop=(kt == KC - 1))
            uvc = p_sm.tile([128, 1], F32, tag=f"uv{mt}", name=f"uv_{mt}")
            nc.vector.tensor_copy(uvc[:], pv[:, 0:1])
            uv_c.append(uvc)
            cvc = p_sm.tile([128, 1], F32, tag=f"cv{mt}", name=f"cv_{mt}")
            nc.vector.tensor_add(cvc[:], pv[:, 1:2], bv_c[mt][:])
            cv_c.append(cvc)
            ut = p_sm.tile([128, H], F32, tag=f"u{mt}", name=f"u_{mt}")
            nc.vector.tensor_scalar_mul(ut[:], mask_t[mt][:], ukc[:])
            u_t.append(ut)
        for mt in range(KC):
            pm = ps_x.tile([128, H], F32, tag="small", name=f"pm{mt}")
            for kt in range(KC):
                nc.tensor.matmul(pm[:], wq_t[kt][:, mt * 128:(mt + 1) * 128],
                                 u_t[kt][:], start=(kt == 0), stop=(kt == KC - 1))
            mt_sb = p_sm.tile([128, H], F32R, tag=f"m{mt}", name=f"m_{mt}")
            nc.vector.tensor_copy(mt_sb[:], pm[:])
            m_t.append(mt_sb)
        pa0 = ps_x.tile([H, 1], F32, tag="small", name="pa0")
        for kt in range(KC):
            nc.tensor.matmul(pa0[:], u_t[kt][:], bq_c[kt][:],
                             start=(kt == 0), stop=(kt == KC - 1))
        a0s = p_sm.tile([H, 1], F32, tag="a0s")
        nc.scalar.mul(a0s[:], pa0[:], SCALE / SCAL)

        # ---- a path: tt = a/SCAL in [H, T]; repack to [128, 32] ----------
        pa = ps_x.tile([H, T], F32, tag="small", name="pa")
        for kt in range(KC):
            nc.tensor.matmul(pa[:], m_t[kt][:], xs_t[kt][:],
                             start=(kt == 0), stop=(kt == KC - 1))
        tt_sb = p_sm.tile([H, T], F32, tag="tts")
        nc.scalar.activation(tt_sb[:], pa[:], AF.Identity, bias=a0s[:],
                             scale=SCALE / SCAL)
        tt = p_cl.tile([128, FP], F32, tag="tt")
        for b in range(B):
            src = tt_sb[:, b * LC:(b + 1) * LC].rearrange(
                "h (lhi llo) -> h lhi llo", llo=FP)
            nc.sync.dma_start(tt[b * 64:(b + 1) * 64, :], src)
        nc.vector.tensor_scalar_max(tt[:], tt[:], -1.0)
        nc.vector.tensor_scalar_min(tt[:], tt[:], 1.0)

        # ---- both-batch softmax collapse at 64 Chebyshev nodes -----------
        e_b = p_node.tile([128, G], F32, tag="ndA")
        for b in range(B):
            nc.sync.dma_start(e_b[b * 64:(b + 1) * 64, :],
                              expv[b, :][None, :].to_broadcast((64, G)))
        pn = p_node.tile([128, G], F32, tag="ndB")
        z_col = p_sm.tile([128, 1], F32, tag="zc")
        nc.scalar.activation(pn[:], e_b[:], AF.Exp, scale=xn_col[:],
                             accum_out=z_col[:])
        nm_col = p_sm.tile([128, 1], F32, tag="nmc")
        nc.vector.scalar_tensor_tensor(
            out=pn[:], in0=pn[:], scalar=1.0, in1=e_b[:],
            op0=OP.mult, op1=OP.mult, accum_out=nm_col[:])
        zr_col = p_sm.tile([128, 1], F32, tag="zrc")
        nc.vector.reciprocal(zr_col[:], z_col[:])
        f_col = p_sm.tile([128, 1], F32, tag="fc")
        nc.vector.tensor_mul(f_col[:], nm_col[:], zr_col[:])
        pck = ps_x.tile([2 * KDEG, 1], F32, tag="small", name="pck")
        nc.tensor.matmul(pck[:], dct_sb[:], f_col[:], start=True, stop=True)
        ck_sb = p_sm.tile([2 * KDEG, 1], F32, tag="cksb")
        nc.vector.tensor_copy(ck_sb[:], pck[:])
        nc.sync.dma_start(ck_dram[:].rearrange("b k -> (b k)"), ck_sb[:])
        # broadcast coeffs to the pack layout: cb[p, k] = ck[b(p), k]
        cb = p_cl.tile([128, KDEG], F32, tag="cb")
        nc.sync.dma_start(
            cb[:], ck_dram[:, None, :].to_broadcast((B, 64, KDEG)))

        # ---- Clenshaw over packed a: [128, 32], p = b*64 + h*8 + lhi -----
        tt2 = p_cl.tile([128, FP], F32, tag="tt2")
        nc.vector.tensor_add(tt2[:], tt[:], tt[:])
        bb1 = p_cl.tile([128, FP], F32, tag="bb1")
        bb2 = p_cl.tile([128, FP], F32, tag="bb2")
        tmp = p_cl.tile([128, FP], F32, tag="tmp")
        nc.vector.memset(bb1[:], 0.0)
        nc.vector.memset(bb2[:], 0.0)
        cur1, cur2 = bb1, bb2
        for k in range(KDEG - 1, 0, -1):
            # b_new = (2t*b1 + c_k) - b2 ; write into cur2, then swap
            nc.vector.tensor_mul(tmp[:], tt2[:], cur1[:])
            nc.vector.scalar_tensor_tensor(
                out=cur2[:], in0=tmp[:], scalar=cb[:, k:k + 1], in1=cur2[:],
                op0=OP.add, op1=OP.subtract)
            cur1, cur2 = cur2, cur1
        w_pack = p_cl.tile([128, FP], F32, tag="wp")
        nc.vector.tensor_mul(tmp[:], tt[:], cur1[:])
        nc.vector.scalar_tensor_tensor(
            out=w_pack[:], in0=tmp[:], scalar=cb[:, 0:1], in1=cur2[:],
            op0=OP.add, op1=OP.subtract)
        nc.sync.dma_start(
            w_dram[:].rearrange("b h (lhi llo) -> (b h lhi) llo", llo=FP),
            w_pack[:])

        # ---- x_attn + residual -> y -------------------------------------
        y_t = []
        for kt in range(KC):
            wr = p_act.tile([128, T], F32, tag="wrep", bufs=2, name=f"wr{kt}")
            for j in range(2):
                hh = 2 * kt + j
                nc.scalar.dma_start(
                    wr[64 * j:64 * (j + 1), :],
                    w_dram[:, hh, :][None, :, :].to_broadcast((64, B, LC)))
            xa = p_act.tile([128, T], F32, tag="tmpx", bufs=2, name=f"xa{kt}")
            nc.vector.tensor_scalar(xa[:], wr[:], uv_c[kt][:], cv_c[kt][:],
                                    op0=OP.mult, op1=OP.add)
            yk = p_act.tile([128, T], FP16, tag="y", name=f"y{kt}")
            nc.vector.tensor_add(yk[:], xa[:], xs_t[kt][:].bitcast(F32))
            y_t.append(yk)

        def layernorm(y_tiles, g_row, be_cols, out_tag, out_pool, ph,
                      out_bufs=None):
            # mu = ones(1/C).T @ y ; msq = ones(1/C).T @ y^2  (fp16 matmuls)
            stat0 = ps_x.tile([1, T], F32, tag="st0", name=f"st0{ph}")
            stat1 = ps_x.tile([1, T], F32, tag="st1", name=f"st1{ph}")
            for kt in range(KC):
                nc.tensor.matmul(stat0[:], onesk_h[:], y_tiles[kt][:],
                                 start=(kt == 0), stop=(kt == KC - 1))
            sq_t = []
            for kt in range(KC):
                sq = p_act.tile([128, T], FP16, tag="sq", bufs=2,
                                name=f"sq{ph}{kt}")
                nc.scalar.activation(sq[:], y_tiles[kt][:], AF.Square)
                sq_t.append(sq)
            for kt in range(KC):
                nc.tensor.matmul(stat1[:], onesk_h[:], sq_t[kt][:],
                                 start=(kt == 0), stop=(kt == KC - 1))
            musq_row = p_sm.tile([1, T], F32, tag="lnrow", bufs=4, name=f"musq{ph}")
            nc.scalar.activation(musq_row[:], stat0[:], AF.Square)
            var_row = p_sm.tile([1, T], F32, tag="lnrow", bufs=4, name=f"var{ph}")
            nc.vector.tensor_sub(var_row[:], stat1[:], musq_row[:])
            std_row = p_sm.tile([1, T], F32, tag="lnrow", bufs=4, name=f"std{ph}")
            nc.scalar.activation(std_row[:], var_row[:], AF.Sqrt, bias=eps_col[:])
            rstd_row = p_sm.tile([1, T], F32R, tag="rstdr", name=f"rstd{ph}")
            with nc.allow_low_precision(reason="f32r feeds full-rate PE matmul"):
                nc.vector.reciprocal(rstd_row[:], std_row[:])
            q_row = p_sm.tile([1, T], F32R, tag="qr", name=f"q{ph}")
            nc.vector.tensor_mul(q_row[:], stat0[:], rstd_row[:].bitcast(F32))
            outs = []
            for kt in range(KC):
                sl = slice(kt * 128, (kt + 1) * 128)
                pA = ps_mm.tile([128, T], F32, tag="mm", name=f"pA{ph}{kt}")
                nc.tensor.matmul(pA[:], g_row[0:1, sl], rstd_row[:],
                                 start=True, stop=True)
                pB = ps_mm.tile([128, T], F32, tag="mm", name=f"pB{ph}{kt}")
                nc.tensor.matmul(pB[:], g_row[0:1, sl], q_row[:],
                                 start=True, stop=True)
                tx = p_act.tile([128, T], F32, tag="tmpx", bufs=2,
                                name=f"tx{ph}{kt}")
                nc.vector.tensor_mul(tx[:], y_tiles[kt][:], pA[:])
                xo = out_pool.tile([128, T], FP16, tag=out_tag,
                                   bufs=out_bufs, name=f"ln{ph}{kt}")
                # xo = (tx + beta) - g*mu*rstd
                nc.vector.scalar_tensor_tensor(
                    out=xo[:], in0=tx[:], scalar=be_cols[kt][:], in1=pB[:],
                    op0=OP.add, op1=OP.subtract)
                outs.append(xo)
            return outs

        x_t = layernorm(y_t, g1_row, be1_c, "x", p_act, "a")

        # ---- FFN1: h = relu(W1 @ x + b1) ---------------------------------
        h_t = []
        for mt in range(KH):
            sl = slice(mt * 128, (mt + 1) * 128)
            pf = ps_mm.tile([128, T], F32, tag="mm", name=f"pf1{mt}")
            for kt in range(KC):
                nc.tensor.matmul(pf[:], w1_t[kt][:, sl], x_t[kt][:],
                                 start=(kt == 0), stop=(kt == KC - 1))
            hm = p_kvh.tile([128, T], FP16, tag="h", bufs=16, name=f"h{mt}")
            nc.scalar.activation(hm[:], pf[:], AF.Relu, bias=b1_c[mt][:])
            h_t.append(hm)

        # ---- FFN2 + residual -> y2 ---------------------------------------
        y2_t = []
        for mt in range(KC):
            sl = slice(mt * 128, (mt + 1) * 128)
            pf = ps_mm.tile([128, T], F32, tag="mm", name=f"pf2{mt}")
            for kt in range(KH):
                nc.tensor.matmul(pf[:], w2_t[kt][:, sl], h_t[kt][:],
                                 start=(kt == 0), stop=(kt == KH - 1))
            y2 = p_act.tile([128, T], FP16, tag="y", name=f"y2{mt}")
            # y2 = (x + b2) + psum
            nc.vector.scalar_tensor_tensor(
                out=y2[:], in0=x_t[mt][:], scalar=b2_c[mt][:],
                in1=pf[:], op0=OP.add, op1=OP.add)
            y2_t.append(y2)

        z_t = layernorm(y2_t, g2_row, be2_c, "z", p_xsz, "b")

        # ---- output proj: out = Wo @ z + bo ------------------------------
        for mt in range(KC):
            sl = slice(mt * 128, (mt + 1) * 128)
            pf = ps_mm.tile([128, T], F32, tag="mm", name=f"pfo{mt}")
            for kt in range(KC):
                nc.tensor.matmul(pf[:], wo_t[kt][:, sl], z_t[kt][:],
                                 start=(kt == 0), stop=(kt == KC - 1))
            om = p_act.tile([128, T], F32, tag="tmpx", bufs=2, name=f"om{mt}")
            nc.scalar.activation(om[:], pf[:], AF.Identity, bias=bo_c[mt][:])
            for b in range(B):
                nc.scalar.dma_start(out_sl[b, mt * 128:(mt + 1) * 128, :],
                                  om[:, b * LC:(b + 1) * LC])

    nc.compile()
    return nc


def kernel(**inputs):
    global _CACHE, LAST_RESULTS
    if _CACHE is None:
        _CACHE = _build()
    nc = _CACHE

    f32 = lambda x: np.ascontiguousarray(np.asarray(x), dtype=np.float32)
    f16t = lambda x: np.ascontiguousarray(np.asarray(x).T, dtype=np.float16)
    seq = f32(inputs["seq"])
    base = {
        "expv": f32(inputs["exp"]),
        "wq": f32(inputs["Wq"]),
        "wkt": f32(np.asarray(inputs["Wk"]).T),
        "wvt": f32(np.asarray(inputs["Wv"]).T),
        "w1t": f16t(inputs["W1"]),
        "w2t": f16t(inputs["W2"]),
        "wot": f16t(inputs["Wo"]),
        "wg": f32(inputs["Wg"]),
        "bgv": f32(inputs["bg"]),
        "bqv": f32(inputs["bq"]),
        "bvv": f32(inputs["bv"]),
        "b1v": f32(inputs["b1"]),
        "b2v": f32(inputs["b2"]),
        "bov": f32(inputs["bo"]),
        "g1v": f32(inputs["g1"]),
        "be1": f32(inputs["beta1"]),
        "g2v": f32(inputs["g2"]),
        "be2": f32(inputs["beta2"]),
    }
    in_maps = []
    for c in range(NCORES):
        m = dict(base)
        m["seq_sl"] = np.ascontiguousarray(seq[:, :, c * LC:(c + 1) * LC])
        in_maps.append(m)

    res = run_bass_kernel_spmd(nc, in_maps, list(range(NCORES)), trace=TRACE,
                               **TRACE_KW)
    LAST_RESULTS = res
    out = np.empty((B, C, L), np.float32)
    for c in range(NCORES):
        out[:, :, c * LC:(c + 1) * LC] = res.results[c]["out_sl"]
    return out



# revision 18
# speedup vs baseline: 1.2412x; 1.2412x over previous
"""Trainium2 Bass kernel for nn_G3DCrossAttention (B=2, C=512, L=2048, G=2048, H=8).

Same exact-math rank-1 collapse as the baseline (see kernel_v1_baseline.py), with
the attention reduced to w = f_b(a) evaluated via a 64-node Chebyshev fit and a
Clenshaw recurrence. This version restructures for latency:
  - stage A (u_k/u_v/c_v/M) computed as f32r row-major matmuls with N=512 free
    dims instead of ~104 serialized fp32 LOW_HIGH N<=8 matmuls (38us -> ~5us)
  - all small constant loads batched host-side into 3 packed tensors (one DMA
    each) instead of ~60 individual dma_starts (~650ns issue cost each)
  - big weights loaded with one DMA per tensor via partition-folding rearranges
  - the Chebyshev-coefficient broadcast is a block-ones matmul in SBUF/PSUM
    instead of a DRAM round trip (~12us of DMA latency removed)
  - the per-head w broadcast is a selector matmul from a [H,T] tile (plus the
    c_v term accumulated as a K=1 matmul) instead of DRAM round trip + 8
    broadcast DMAs
  - LN rstd uses ACT Rsqrt instead of Sqrt + single-lane reciprocal (3.3us each)
  - KDEG=16 (w err ~4e-4, full-pipeline fp32 err 2.5e-4, gate is 2e-2)

Sharding: data-parallel over L across 8 cores (L/8 = 256 queries each).
"""

from contextlib import ExitStack

import numpy as np

import concourse.bass as bass
import concourse.tile as tile
from concourse import bacc, mybir
from concourse.bass_utils import run_bass_kernel_spmd

F32 = mybir.dt.float32
F32R = mybir.dt.float32r
FP16 = mybir.dt.float16
AF = mybir.ActivationFunctionType
OP = mybir.AluOpType

B, C, L, G, H = 2, 512, 2048, 2048, 8
D = C // H
NCORES = 8
LC = L // NCORES              # 256 queries per core
T = B * LC                    # 512 tokens per core (tau = b*LC + l)
KC = C // 128                 # 4 partition tiles over C
KH = (4 * C) // 128           # 16 partition tiles over 4C
FP = LC // 8                  # 32: free dim of the packed a/w tiles
SCALE = 1.0 / float(np.sqrt(D))
EPS = 1e-5
SCAL = 5.0                    # Chebyshev half-range in a-units (|a|max ~ 4.43)
KDEG = 16                     # Chebyshev series length
MNODES = 64                   # Chebyshev nodes per batch (2 batches -> 128 parts)
NPC = 7                       # packed per-kt columns: wg bg bq bo b2 be1 be2

TRACE = False
TRACE_KW = {}
LAST_RESULTS = None
DBG = False

_CACHE = None


def _consts():
    m = np.arange(MNODES)
    theta = np.pi * (2 * m + 1) / (2 * MNODES)
    xn64 = (SCAL * np.cos(theta)).astype(np.float32)
    xnodes = np.concatenate([xn64, xn64])                 # [128] both batches
    dct1 = np.zeros((MNODES, KDEG), np.float32)
    for k in range(KDEG):
        dct1[:, k] = (2.0 / MNODES) * np.cos(k * theta)
    dct1[:, 0] *= 0.5
    dct_full = np.concatenate([dct1, dct1], axis=0)       # [128, KDEG]
    maskc = np.zeros((C, H), np.float32)
    for h in range(H):
        maskc[h * D:(h + 1) * D, h] = 1.0
    mask128 = maskc.reshape(KC, 128, H).transpose(1, 0, 2).reshape(128, KC * H)
    blockones = np.zeros((128, 128), np.float32)
    blockones[:64, :64] = 1.0
    blockones[64:, 64:] = 1.0
    # cst layout: [dct KDEG][xn 1][mask KC*H][blockones 128]
    cst = np.concatenate(
        [dct_full, xnodes[:, None], mask128, blockones], axis=1)
    sel = np.zeros((H, C), np.float32)                    # sel[h, c] = [c//D == h]
    for h in range(H):
        sel[h, h * D:(h + 1) * D] = 1.0
    return cst, sel


def _build():
    nc = bacc.Bacc(debug=False, num_devices=NCORES)

    # ---- external inputs -------------------------------------------------
    seq_sl = nc.dram_tensor("seq_sl", [B, C, LC], F32, kind="ExternalInput")
    expv = nc.dram_tensor("expv", [B, G], F32, kind="ExternalInput")
    wq = nc.dram_tensor("wq", [C, C], F32, kind="ExternalInput")        # Wq as stored
    wkt = nc.dram_tensor("wkt", [C, C], F32, kind="ExternalInput")      # Wk.T
    wvt = nc.dram_tensor("wvt", [C, C], F32, kind="ExternalInput")      # Wv.T
    w1t = nc.dram_tensor("w1t", [C, 4 * C], FP16, kind="ExternalInput")  # W1.T fp16
    w2t = nc.dram_tensor("w2t", [4 * C, C], FP16, kind="ExternalInput")  # W2.T fp16
    wot = nc.dram_tensor("wot", [C, C], FP16, kind="ExternalInput")      # Wo.T fp16
    pcd = nc.dram_tensor("pcd", [KC, 128, NPC], F32, kind="ExternalInput")
    b1d = nc.dram_tensor("b1d", [KH, 128], F32, kind="ExternalInput")
    rowsd = nc.dram_tensor("rowsd", [3, C], F32, kind="ExternalInput")  # bv g1 g2

    out_sl = nc.dram_tensor("out_sl", [B, C, LC], F32, kind="ExternalOutput")
    dbg = {}
    if DBG:
        for nm, shp in [("d_ukc", [128, KC]), ("d_uvc", [128, KC]),
                        ("d_cvT", [1, C]), ("d_ut", [128, KC * H]),
                        ("d_m", [128, KC * H]), ("d_tts", [H, T]),
                        ("d_tt", [128, FP]), ("d_cb", [128, KDEG]),
                        ("d_wp", [128, FP]), ("d_wht", [H, T]),
                        ("d_y", [128, KC * T]), ("d_x", [128, KC * T])]:
            dbg[nm] = nc.dram_tensor(nm, shp, F32, kind="ExternalOutput")
    w_dram = nc.dram_tensor("w_scr", [B, H, LC], F32)     # fallback repack scratch
    ukuv_d = nc.dram_tensor("ukuv_scr", [2, C], F32)

    cst_np, sel_np = _consts()
    c_cst = nc.inline_tensor(cst_np, name="c_cst")
    c_sel = nc.inline_tensor(sel_np, name="c_sel")
    c_onesk = nc.inline_tensor(np.full((128, 1), 1.0 / C, np.float16),
                               name="c_onesk")
    c_ones = nc.inline_tensor(np.ones((1, B * LC), np.float32), name="c_ones")

    NCST = cst_np.shape[1]
    O_DCT, O_XN, O_MASK, O_BLK = 0, KDEG, KDEG + 1, KDEG + 1 + KC * H

    with tile.TileContext(nc) as tc, ExitStack() as ctx:
        p_big = ctx.enter_context(tc.tile_pool(name="big", bufs=1))
        p_act = ctx.enter_context(tc.tile_pool(name="act", bufs=4))
        p_sm = ctx.enter_context(tc.tile_pool(name="sm", bufs=1))
        p_cl = ctx.enter_context(tc.tile_pool(name="cl", bufs=1))
        ps_mm = ctx.enter_context(tc.tile_pool(name="psmm", bufs=4, space="PSUM"))
        ps_x = ctx.enter_context(tc.tile_pool(name="psx", bufs=4, space="PSUM"))

        # ---- critical loads on the sync queue ----------------------------
        def load_fold(tile_ap, dram_t, pat_src, pat_dst, eng, **kw):
            nc_eng = getattr(nc, eng)
            nc_eng.dma_start(tile_ap.rearrange(pat_dst, **kw),
                             dram_t.rearrange(pat_src, p=128, **kw))

        wkt_sb = p_big.tile([128, KC * C], F32R, tag="wkt")
        load_fold(wkt_sb[:], wkt.bitcast(F32R), "(kt p) c -> p kt c",
                  "p (kt c) -> p kt c", "sync", kt=KC)
        wvt_sb = p_big.tile([128, KC * C], F32R, tag="wvt")
        load_fold(wvt_sb[:], wvt.bitcast(F32R), "(kt p) c -> p kt c",
                  "p (kt c) -> p kt c", "sync", kt=KC)
        wq_sb = p_big.tile([128, KC * C], F32R, tag="wq")
        load_fold(wq_sb[:], wq.bitcast(F32R), "(kt p) c -> p kt c",
                  "p (kt c) -> p kt c", "sync", kt=KC)
        xs = p_big.tile([128, KC * T], F32R, tag="xs")
        for b in range(B):
            nc.sync.dma_start(
                xs[:].rearrange("p (kt b l) -> p kt b l", kt=KC, b=B)[:, :, b, :],
                seq_sl[b].rearrange("(kt p) l -> p kt l", p=128).bitcast(F32R))

        # ---- packed smalls (scalar queue) --------------------------------
        cst_sb = p_sm.tile([128, NCST], F32R, tag="cst")
        nc.scalar.dma_start(cst_sb[:], c_cst[:].bitcast(F32R))
        sel_sb = p_sm.tile([H, C], F32R, tag="sel")
        nc.scalar.dma_start(sel_sb[:], c_sel[:].bitcast(F32R))
        onesk_h = p_sm.tile([128, 1], FP16, tag="onesk")
        nc.scalar.dma_start(onesk_h[:], c_onesk[:])
        pc = p_sm.tile([128, KC * NPC], F32R, tag="pc")
        nc.scalar.dma_start(pc[:].rearrange("p (kt n) -> p kt n", kt=KC),
                            pcd.rearrange("kt p n -> p kt n").bitcast(F32R))
        b1c = p_sm.tile([128, KH], F32, tag="b1c")
        nc.scalar.dma_start(b1c[:], b1d.rearrange("kh p -> p kh"))
        rows_sb = p_sm.tile([1, 3 * C], F32R, tag="rows")
        nc.scalar.dma_start(rows_sb[:],
                            rowsd.rearrange("r c -> (r c)")[None, :].bitcast(F32R))
        e_b = p_cl.tile([128, G], F32, tag="eb")
        for b in range(B):
            nc.scalar.dma_start(e_b[b * 64:(b + 1) * 64, :],
                                expv[b, :][None, :].to_broadcast((64, G)))

        # ---- bulk fp16 weights on the gpsimd queue -----------------------
        w1_sb = p_big.tile([128, KC * 4 * C], FP16, tag="w1")
        load_fold(w1_sb[:], w1t, "(kt p) m -> p kt m", "p (kt m) -> p kt m",
                  "gpsimd", kt=KC)
        w2_sb = p_big.tile([128, KH * C], FP16, tag="w2")
        load_fold(w2_sb[:], w2t, "(kh p) c -> p kh c", "p (kh c) -> p kh c",
                  "gpsimd", kh=KH)
        wo_sb = p_big.tile([128, KC * C], FP16, tag="wo")
        load_fold(wo_sb[:], wot, "(kt p) c -> p kt c", "p (kt c) -> p kt c",
                  "gpsimd", kt=KC)

        def pccol_r(kt, j, n=1):
            return pc[:, kt * NPC + j:kt * NPC + j + n]

        def pccol(kt, j, n=1):
            return pccol_r(kt, j, n).bitcast(F32)

        eps_col = p_sm.tile([1, 1], F32, tag="epsc")
        nc.vector.memset(eps_col[:], EPS)
        ones_row = p_sm.tile([1, T], F32R, tag="ones")
        nc.scalar.dma_start(ones_row[:], c_ones[:].bitcast(F32R))

        # ---- stage A: u_k, u_v, c_v as rows (f32r, N=512) ----------------
        puk = ps_x.tile([1, C], F32, tag="x", name="puk")
        puv = ps_x.tile([1, C], F32, tag="x", name="puv")
        pcv = ps_x.tile([1, C], F32, tag="x", name="pcv")
        for kt in range(KC):
            nc.tensor.matmul(puk[:], pccol_r(kt, 0),
                             wkt_sb[:, kt * C:(kt + 1) * C],
                             start=(kt == 0), stop=(kt == KC - 1))
        for kt in range(KC):
            nc.tensor.matmul(puv[:], pccol_r(kt, 0),
                             wvt_sb[:, kt * C:(kt + 1) * C],
                             start=(kt == 0), stop=(kt == KC - 1))
        for kt in range(KC):
            nc.tensor.matmul(pcv[:], pccol_r(kt, 1),
                             wvt_sb[:, kt * C:(kt + 1) * C],
                             start=(kt == 0), stop=(kt == KC - 1))
        ukT = p_sm.tile([1, C], F32, tag="ukT")
        nc.vector.tensor_copy(ukT[:], puk[:])
        uvT = p_sm.tile([1, C], F32, tag="uvT")
        nc.vector.tensor_copy(uvT[:], puv[:])
        cvT = p_sm.tile([1, C], F32R, tag="cvT")
        nc.vector.tensor_add(cvT[:], pcv[:], rows_sb[0:1, 0:C].bitcast(F32))

        # columns for per-partition scalar use
        nc.sync.dma_start(ukuv_d[0][None, :], ukT[:])
        nc.sync.dma_start(ukuv_d[1][None, :], uvT[:])
        uk_cols = p_sm.tile([128, KC], F32, tag="ukc")
        nc.sync.dma_start(uk_cols[:], ukuv_d[0].rearrange("(kt p) -> p kt", p=128))
        uv_cols = p_sm.tile([128, KC], F32, tag="uvc")
        nc.sync.dma_start(uv_cols[:], ukuv_d[1].rearrange("(kt p) -> p kt", p=128))

        if DBG:
            nc.sync.dma_start(dbg["d_ukc"][:], uk_cols[:])
            nc.sync.dma_start(dbg["d_uvc"][:], uv_cols[:])
            nc.sync.dma_start(dbg["d_cvT"][:], cvT[:].bitcast(F32))
        u_t = []
        for kt in range(KC):
            ut = p_sm.tile([128, H], F32R, tag=f"u{kt}", name=f"u_{kt}")
            nc.vector.tensor_scalar_mul(
                ut[:], cst_sb[:, O_MASK + kt * H:O_MASK + (kt + 1) * H].bitcast(F32),
                uk_cols[:, kt:kt + 1])
            u_t.append(ut)

        # ---- M = Wq.T-contracted masked-u; a0 ----------------------------
        m_t = []
        for mt in range(KC):
            pm = ps_x.tile([128, H], F32, tag="x", name=f"pm{mt}")
            for kt in range(KC):
                sl = slice(kt * C + mt * 128, kt * C + (mt + 1) * 128)
                nc.tensor.matmul(pm[:], wq_sb[:, sl], u_t[kt][:],
                                 start=(kt == 0), stop=(kt == KC - 1))
            mt_sb = p_sm.tile([128, H], F32R, tag=f"m{mt}", name=f"m_{mt}")
            nc.vector.tensor_copy(mt_sb[:], pm[:])
            m_t.append(mt_sb)
        if DBG:
            for kt in range(KC):
                nc.sync.dma_start(dbg["d_ut"][:, kt * H:(kt + 1) * H],
                                  u_t[kt][:].bitcast(F32))
                nc.sync.dma_start(dbg["d_m"][:, kt * H:(kt + 1) * H],
                                  m_t[kt][:].bitcast(F32))
        pa0 = ps_x.tile([H, 2], F32, tag="x", name="pa0")
        for kt in range(KC):
            nc.tensor.matmul(pa0[:], u_t[kt][:], pccol_r(kt, 2, 2),
                             start=(kt == 0), stop=(kt == KC - 1))
        a0s = p_sm.tile([H, 1], F32, tag="a0s")
        nc.scalar.mul(a0s[:], pa0[:, 0:1], SCALE / SCAL)

        # ---- a path: tt = a/SCAL in [H, T]; repack to [128, 32] ----------
        pa = ps_x.tile([H, T], F32, tag="x", name="pa")
        for kt in range(KC):
            nc.tensor.matmul(pa[:], m_t[kt][:], xs[:, kt * T:(kt + 1) * T],
                             start=(kt == 0), stop=(kt == KC - 1))
        tt_sb = p_sm.tile([H, T], F32, tag="tts")
        nc.scalar.activation(tt_sb[:], pa[:], AF.Identity, bias=a0s[:],
                             scale=SCALE / SCAL)
        tt = p_cl.tile([128, FP], F32, tag="tt")
        for b in range(B):
            src = tt_sb[:, b * LC:(b + 1) * LC].rearrange(
                "h (lhi llo) -> h lhi llo", llo=FP)
            nc.sync.dma_start(tt[b * 64:(b + 1) * 64, :], src)
        nc.vector.tensor_scalar_max(tt[:], tt[:], -1.0)
        nc.vector.tensor_scalar_min(tt[:], tt[:], 1.0)
        if DBG:
            nc.sync.dma_start(dbg["d_tts"][:], tt_sb[:])
            nc.sync.dma_start(dbg["d_tt"][:], tt[:])

        # ---- both-batch softmax collapse at 64 Chebyshev nodes -----------
        pn = p_cl.tile([128, G], F32, tag="ndB")
        z_col = p_sm.tile([128, 1], F32, tag="zc")
        nc.scalar.activation(pn[:], e_b[:], AF.Exp,
                             scale=cst_sb[:, O_XN:O_XN + 1].bitcast(F32),
                             accum_out=z_col[:])
        nm_col = p_sm.tile([128, 1], F32, tag="nmc")
        nc.vector.scalar_tensor_tensor(
            out=pn[:], in0=pn[:], scalar=1.0, in1=e_b[:],
            op0=OP.mult, op1=OP.mult, accum_out=nm_col[:])
        zr_col = p_sm.tile([128, 1], F32, tag="zrc")
        nc.vector.reciprocal(zr_col[:], z_col[:])
        f_col = p_sm.tile([128, 1], F32, tag="fc")
        nc.vector.tensor_mul(f_col[:], nm_col[:], zr_col[:])

        # cb[p, k] = ck[batch(p), k] via block-ones matmul (no DRAM trip)
        fdct = p_sm.tile([128, KDEG], F32R, tag="fdct")
        nc.vector.tensor_scalar_mul(
            fdct[:], cst_sb[:, O_DCT:O_DCT + KDEG].bitcast(F32), f_col[:])
        pcb = ps_x.tile([128, KDEG], F32, tag="x", name="pcb")
        nc.tensor.matmul(pcb[:], cst_sb[:, O_BLK:O_BLK + 128], fdct[:],
                         start=True, stop=True)
        cb = p_cl.tile([128, KDEG], F32, tag="cb")
        nc.vector.tensor_copy(cb[:], pcb[:])

        if DBG:
            nc.sync.dma_start(dbg["d_cb"][:], cb[:])
        # ---- Clenshaw over packed a: [128, 32] ---------------------------
        tt2 = p_cl.tile([128, FP], F32, tag="tt2")
        nc.vector.tensor_add(tt2[:], tt[:], tt[:])
        bb1 = p_cl.tile([128, FP], F32, tag="bb1")
        bb2 = p_cl.tile([128, FP], F32, tag="bb2")
        tmp = p_cl.tile([128, FP], F32, tag="tmp")
        nc.vector.memset(bb1[:], 0.0)
        nc.vector.memset(bb2[:], 0.0)
        cur1, cur2 = bb1, bb2
        for k in range(KDEG - 1, 0, -1):
            nc.vector.tensor_mul(tmp[:], tt2[:], cur1[:])
            nc.vector.scalar_tensor_tensor(
                out=cur2[:], in0=tmp[:], scalar=cb[:, k:k + 1], in1=cur2[:],
                op0=OP.add, op1=OP.subtract)
            cur1, cur2 = cur2, cur1
        w_pack = p_cl.tile([128, FP], F32, tag="wp")
        nc.vector.tensor_mul(tmp[:], tt[:], cur1[:])
        nc.vector.scalar_tensor_tensor(
            out=w_pack[:], in0=tmp[:], scalar=cb[:, 0:1], in1=cur2[:],
            op0=OP.add, op1=OP.subtract)

        if DBG:
            nc.sync.dma_start(dbg["d_wp"][:], w_pack[:])
        # ---- w to [H, T] layout (SBUF->SBUF partition repack) ------------
        w_HT = p_sm.tile([H, T], F32R, tag="wht")
        for b in range(B):
            dst = w_HT[:, b * LC:(b + 1) * LC].rearrange(
                "h (lhi llo) -> h lhi llo", llo=FP)
            nc.sync.dma_start(dst, w_pack[b * 64:(b + 1) * 64, :].bitcast(F32R))

        if DBG:
            nc.sync.dma_start(dbg["d_wht"][:], w_HT[:].bitcast(F32))
        # ---- x_attn + residual -> y via selector matmul ------------------
        y_t = []
        for mt in range(KC):
            wr = ps_mm.tile([128, T], F32, tag="mm", name=f"wr{mt}")
            nc.tensor.matmul(wr[:], sel_sb[:, mt * 128:(mt + 1) * 128],
                             w_HT[:], start=True, stop=False)
            nc.tensor.matmul(wr[:], cvT[:, mt * 128:(mt + 1) * 128],
                             ones_row[:], start=False, stop=True)
            yk = p_act.tile([128, T], FP16, tag="y", name=f"y{mt}")
            # yk = (wr * uv) + xs   (wr already contains w_bcast + cv)
            nc.vector.scalar_tensor_tensor(
                out=yk[:], in0=wr[:], scalar=uv_cols[:, mt:mt + 1],
                in1=xs[:, mt * T:(mt + 1) * T].bitcast(F32),
                op0=OP.mult, op1=OP.add)
            y_t.append(yk)

        if DBG:
            for mt in range(KC):
                nc.gpsimd.dma_start(dbg["d_y"][:, mt * T:(mt + 1) * T],
                                     y_t[mt][:])
        g1_row = rows_sb[0:1, C:2 * C]
        g2_row = rows_sb[0:1, 2 * C:3 * C]

        def layernorm(y_tiles, g_row, becol_j, ph):
            stat0 = ps_x.tile([1, T], F32, tag="x", name=f"st0{ph}")
            stat1 = ps_x.tile([1, T], F32, tag="x", name=f"st1{ph}")
            for kt in range(KC):
                nc.tensor.matmul(stat0[:], onesk_h[:], y_tiles[kt][:],
                                 start=(kt == 0), stop=(kt == KC - 1))
            sq_t = []
            for kt in range(KC):
                sq = p_act.tile([128, T], FP16, tag="sq", bufs=2,
                                name=f"sq{ph}{kt}")
                nc.scalar.activation(sq[:], y_tiles[kt][:], AF.Square)
                sq_t.append(sq)
            for kt in range(KC):
                nc.tensor.matmul(stat1[:], onesk_h[:], sq_t[kt][:],
                                 start=(kt == 0), stop=(kt == KC - 1))
            musq_row = p_sm.tile([1, T], F32, tag="lnrow", bufs=4, name=f"musq{ph}")
            nc.scalar.activation(musq_row[:], stat0[:], AF.Square)
            var_row = p_sm.tile([1, T], F32, tag="lnrow", bufs=4, name=f"var{ph}")
            nc.vector.tensor_sub(var_row[:], stat1[:], musq_row[:])
            lv_row = p_sm.tile([1, T], F32, tag="lnrow", bufs=4, name=f"lv{ph}")
            nc.scalar.activation(lv_row[:], var_row[:], AF.Ln, bias=eps_col[:])
            rstd_f32 = p_sm.tile([1, T], F32, tag="lnrow", bufs=4, name=f"rsf{ph}")
            nc.scalar.activation(rstd_f32[:], lv_row[:], AF.Exp, scale=-0.5)
            rstd_row = p_sm.tile([1, T], F32R, tag="lnrow", bufs=4, name=f"rstd{ph}")
            nc.vector.tensor_copy(rstd_row[:], rstd_f32[:])
            q_row = p_sm.tile([1, T], F32R, tag="lnrow", bufs=4, name=f"q{ph}")
            nc.vector.tensor_mul(q_row[:], stat0[:], rstd_f32[:])
            outs = []
            for kt in range(KC):
                sl = slice(kt * 128, (kt + 1) * 128)
                pA = ps_mm.tile([128, T], F32, tag="mm", name=f"pA{ph}{kt}")
                nc.tensor.matmul(pA[:], g_row[0:1, sl], rstd_row[:],
                                 start=True, stop=True)
                pB = ps_mm.tile([128, T], F32, tag="mm", name=f"pB{ph}{kt}")
                nc.tensor.matmul(pB[:], g_row[0:1, sl], q_row[:],
                                 start=True, stop=True)
                tx = p_act.tile([128, T], F32, tag="tmpx", bufs=2,
                                name=f"tx{ph}{kt}")
                nc.vector.tensor_mul(tx[:], y_tiles[kt][:], pA[:])
                xo = p_act.tile([128, T], FP16, tag=f"ln{ph}", bufs=4,
                                name=f"ln{ph}{kt}")
                nc.vector.scalar_tensor_tensor(
                    out=xo[:], in0=tx[:], scalar=pccol(kt, becol_j), in1=pB[:],
                    op0=OP.add, op1=OP.subtract)
                outs.append(xo)
            return outs

        x_t = layernorm(y_t, g1_row, 5, "a")

        if DBG:
            for mt in range(KC):
                nc.gpsimd.dma_start(dbg["d_x"][:, mt * T:(mt + 1) * T],
                                     x_t[mt][:])
        # ---- FFN1: h = relu(W1 @ x + b1) ---------------------------------
        h_t = []
        for mt in range(KH):
            pf = ps_mm.tile([128, T], F32, tag="mm", name=f"pf1{mt}")
            for kt in range(KC):
                sl = slice(kt * 4 * C + mt * 128, kt * 4 * C + (mt + 1) * 128)
                nc.tensor.matmul(pf[:], w1_sb[:, sl], x_t[kt][:],
                                 start=(kt == 0), stop=(kt == KC - 1))
            hm = p_big.tile([128, T], FP16, tag="h", bufs=16, name=f"h{mt}")
            nc.scalar.activation(hm[:], pf[:], AF.Relu, bias=b1c[:, mt:mt + 1])
            h_t.append(hm)

        # ---- FFN2 + residual -> y2 ---------------------------------------
        y2_t = []
        for mt in range(KC):
            pf = ps_mm.tile([128, T], F32, tag="mm", name=f"pf2{mt}")
            for kt in range(KH):
                sl = slice(kt * C + mt * 128, kt * C + (mt + 1) * 128)
                nc.tensor.matmul(pf[:], w2_sb[:, sl], h_t[kt][:],
                                 start=(kt == 0), stop=(kt == KH - 1))
            y2 = p_act.tile([128, T], FP16, tag="y", name=f"y2{mt}")
            nc.vector.scalar_tensor_tensor(
                out=y2[:], in0=x_t[mt][:], scalar=pccol(mt, 4),
                in1=pf[:], op0=OP.add, op1=OP.add)
            y2_t.append(y2)

        z_t = layernorm(y2_t, g2_row, 6, "b")

        # ---- output proj: out = Wo @ z + bo ------------------------------
        for mt in range(KC):
            pf = ps_mm.tile([128, T], F32, tag="mm", name=f"pfo{mt}")
            for kt in range(KC):
                sl = slice(kt * C + mt * 128, kt * C + (mt + 1) * 128)
                nc.tensor.matmul(pf[:], wo_sb[:, sl], z_t[kt][:],
                                 start=(kt == 0), stop=(kt == KC - 1))
            om = p_act.tile([128, T], F32, tag="tmpx", bufs=2, name=f"om{mt}")
            nc.scalar.activation(om[:], pf[:], AF.Identity, bias=pccol(mt, 3))
            nc.scalar.dma_start(
                out_sl[:, mt * 128:(mt + 1) * 128, :].rearrange("b c l -> c b l"),
                om[:])

    nc.compile()
    return nc


def kernel(**inputs):
    global _CACHE, LAST_RESULTS
    if _CACHE is None:
        _CACHE = _build()
    nc = _CACHE

    f32 = lambda x: np.ascontiguousarray(np.asarray(x), dtype=np.float32)
    f16t = lambda x: np.ascontiguousarray(np.asarray(x).T, dtype=np.float16)
    seq = f32(inputs["seq"])

    cols = np.stack([f32(inputs[k]).reshape(C) for k in
                     ("Wg", "bg", "bq", "bo", "b2", "beta1", "beta2")],
                    axis=1)                                   # [C, NPC]
    pcd = np.ascontiguousarray(cols.reshape(KC, 128, NPC))
    b1d = f32(inputs["b1"]).reshape(KH, 128)
    rowsd = np.stack([f32(inputs["bv"]), f32(inputs["g1"]), f32(inputs["g2"])])

    base = {
        "expv": f32(inputs["exp"]),
        "wq": f32(inputs["Wq"]),
        "wkt": f32(np.asarray(inputs["Wk"]).T),
        "wvt": f32(np.asarray(inputs["Wv"]).T),
        "w1t": f16t(inputs["W1"]),
        "w2t": f16t(inputs["W2"]),
        "wot": f16t(inputs["Wo"]),
        "pcd": pcd,
        "b1d": np.ascontiguousarray(b1d),
        "rowsd": np.ascontiguousarray(rowsd),
    }
    in_maps = []
    for c in range(NCORES):
        m = dict(base)
        m["seq_sl"] = np.ascontiguousarray(seq[:, :, c * LC:(c + 1) * LC])
        in_maps.append(m)

    res = run_bass_kernel_spmd(nc, in_maps, list(range(NCORES)), trace=TRACE,
                               **TRACE_KW)
    LAST_RESULTS = res
    out = np.empty((B, C, L), np.float32)
    for c in range(NCORES):
        out[:, :, c * LC:(c + 1) * LC] = res.results[c]["out_sl"]
    return out


# revision 19
# speedup vs baseline: 1.5352x; 1.2369x over previous
"""Trainium2 Bass kernel for nn_G3DCrossAttention (B=2, C=512, L=2048, G=2048, H=8).

Exact-math rank-1 collapse of the attention (see kernel_v1_baseline.py for the
derivation): exp_p is rank-1 in channels, so per head the attention output is
x_attn = w*u_v + c_v with w = f_b(a), a = x_seq @ M + a0. f_b is evaluated at
64 Chebyshev nodes on device (exact softmax-collapse over all G genes), fit
with a KDEG-term Chebyshev series and evaluated by a Clenshaw recurrence.

v3 structure (vs the 175us baseline):
  - u_k/u_v/c_v/M/a0 depend only on weights -> precomputed host-side in numpy
    and shipped as packed constants (kills the 38us on-device stage A and 3MB
    of Wq/Wk/Wv DMA traffic)
  - e_b node matrix built by a K=2 block-ones matmul from a [2,G] tile instead
    of a broadcast DMA (whose descriptor generation took 21us to issue)
  - Chebyshev coefficients broadcast by a block-ones matmul (no DRAM trip)
  - per-head w broadcast by a selector matmul from a [H,T] tile; c_v folded in
    as a K=1 matmul; the [H,T] tile comes from a 2-DMA SBUF->SBUF repack
  - LN rstd via ACT Abs_reciprocal_sqrt (40000-bucket table; one table switch
    total) instead of single-lane reciprocal (3.3us) or Ln+Exp (table thrash)
  - all constants packed into a handful of DMAs; fp16 weights one DMA each
  - KDEG=16 (w err ~4e-4; full-pipeline fp32 err 2.5e-4; gate is 2e-2)

Sharding: data-parallel over L across 8 cores (L/8 = 256 queries each).
"""

from contextlib import ExitStack

import numpy as np

import concourse.bass as bass
import concourse.tile as tile
from concourse import bacc, mybir
from concourse.bass_utils import run_bass_kernel_spmd

F32 = mybir.dt.float32
F32R = mybir.dt.float32r
FP16 = mybir.dt.float16
AF = mybir.ActivationFunctionType
OP = mybir.AluOpType
AX = mybir.AxisListType

B, C, L, G, H = 2, 512, 2048, 2048, 8
D = C // H
NCORES = 8
LC = L // NCORES              # 256 queries per core
T = B * LC                    # 512 tokens per core (tau = b*LC + l)
KC = C // 128                 # 4 partition tiles over C
KH = (4 * C) // 128           # 16 partition tiles over 4C
FP = LC // 8                  # 32: free dim of the packed a/w tiles
GC = G // 512                 # 4 chunks over genes
SCALE = 1.0 / float(np.sqrt(D))
EPS = 1e-5
SCAL = 5.0                    # Chebyshev half-range in a-units (|a|max ~ 4.43)
KDEG = 16                     # Chebyshev series length
MNODES = 64                   # Chebyshev nodes per batch (2 batches -> 128 parts)
NPC = 6                       # packed per-kt cols: bo b2 be1 be2 uv misc

TRACE = False
TRACE_KW = {}
LAST_RESULTS = None
DBG = False

_CACHE = None


def _consts():
    m = np.arange(MNODES)
    theta = np.pi * (2 * m + 1) / (2 * MNODES)
    xn64 = (SCAL * np.cos(theta)).astype(np.float32)
    xnodes = np.concatenate([xn64, xn64])                 # [128] both batches
    dct1 = np.zeros((MNODES, KDEG), np.float32)
    for k in range(KDEG):
        dct1[:, k] = (2.0 / MNODES) * np.cos(k * theta)
    dct1[:, 0] *= 0.5
    dct_full = np.concatenate([dct1, dct1], axis=0)       # [128, KDEG]
    blockones = np.zeros((128, 128), np.float32)
    blockones[:64, :64] = 1.0
    blockones[64:, 64:] = 1.0
    # cst layout: [dct KDEG][xn 1][blockones 128]
    cst = np.concatenate([dct_full, xnodes[:, None], blockones], axis=1)
    sel = np.zeros((H, C), np.float32)                    # sel[h, c] = [c//D == h]
    for h in range(H):
        sel[h, h * D:(h + 1) * D] = 1.0
    bh = np.zeros((2, 128), np.float32)                   # batch-half selector
    bh[0, :64] = 1.0
    bh[1, 64:] = 1.0
    return cst, sel, bh


def _build():
    nc = bacc.Bacc(debug=False, num_devices=NCORES)

    # ---- external inputs -------------------------------------------------
    seq_sl = nc.dram_tensor("seq_sl", [B, C, LC], F32, kind="ExternalInput")
    expv = nc.dram_tensor("expv", [B, G], F32, kind="ExternalInput")
    w1t = nc.dram_tensor("w1t", [C, 4 * C], FP16, kind="ExternalInput")  # W1.T fp16
    w2t = nc.dram_tensor("w2t", [4 * C, C], FP16, kind="ExternalInput")  # W2.T fp16
    wot = nc.dram_tensor("wot", [C, C], FP16, kind="ExternalInput")      # Wo.T fp16
    pcd = nc.dram_tensor("pcd", [KC, 128, NPC], F32, kind="ExternalInput")
    md = nc.dram_tensor("md", [KC, 128, H], F32, kind="ExternalInput")   # M tiles
    b1d = nc.dram_tensor("b1d", [KH, 128], F32, kind="ExternalInput")
    rowsd = nc.dram_tensor("rowsd", [3, C], F32, kind="ExternalInput")   # cv g1 g2

    out_sl = nc.dram_tensor("out_sl", [B, C, LC], F32, kind="ExternalOutput")

    cst_np, sel_np, bh_np = _consts()
    c_cst = nc.inline_tensor(cst_np, name="c_cst")
    c_sel = nc.inline_tensor(sel_np, name="c_sel")
    c_bh = nc.inline_tensor(bh_np, name="c_bh")
    c_onesk = nc.inline_tensor(np.full((128, 1), 1.0 / C, np.float16),
                               name="c_onesk")
    c_ones = nc.inline_tensor(np.ones((1, B * LC), np.float32), name="c_ones")

    NCST = cst_np.shape[1]
    O_DCT, O_XN, O_BLK = 0, KDEG, KDEG + 1

    dbg = {}
    if DBG:
        for nm, shp in [("d_tts", [H, T]), ("d_tt", [128, FP]),
                        ("d_cb", [128, KDEG]), ("d_wp", [128, FP]),
                        ("d_wht", [H, T]), ("d_y", [128, KC * T]),
                        ("d_x", [128, KC * T]), ("d_f", [128, 1])]:
            dbg[nm] = nc.dram_tensor(nm, shp, F32, kind="ExternalOutput")

    with tile.TileContext(nc) as tc, ExitStack() as ctx:
        p_big = ctx.enter_context(tc.tile_pool(name="big", bufs=1))
        p_act = ctx.enter_context(tc.tile_pool(name="act", bufs=4))
        p_sm = ctx.enter_context(tc.tile_pool(name="sm", bufs=1))
        p_cl = ctx.enter_context(tc.tile_pool(name="cl", bufs=1))
        ps_mm = ctx.enter_context(tc.tile_pool(name="psmm", bufs=4, space="PSUM"))
        ps_x = ctx.enter_context(tc.tile_pool(name="psx", bufs=4, space="PSUM"))

        # ---- critical loads (sync queue): xs then expv -------------------
        xs = p_big.tile([128, KC * T], F32R, tag="xs")
        for b in range(B):
            nc.sync.dma_start(
                xs[:].rearrange("p (kt b l) -> p kt b l", kt=KC, b=B)[:, :, b, :],
                seq_sl[b].rearrange("(kt p) l -> p kt l", p=128).bitcast(F32R))
        e2 = p_sm.tile([2, G], F32R, tag="e2")
        nc.sync.dma_start(e2[:], expv[:].bitcast(F32R))

        # ---- packed smalls (scalar queue) --------------------------------
        cst_sb = p_sm.tile([128, NCST], F32R, tag="cst")
        nc.scalar.dma_start(cst_sb[:], c_cst[:].bitcast(F32R))
        m_sb = p_sm.tile([128, KC * H], F32R, tag="msb")
        nc.scalar.dma_start(m_sb[:].rearrange("p (kt h) -> p kt h", kt=KC),
                            md.rearrange("kt p h -> p kt h").bitcast(F32R))
        pc = p_sm.tile([128, KC * NPC], F32R, tag="pc")
        nc.scalar.dma_start(pc[:].rearrange("p (kt n) -> p kt n", kt=KC),
                            pcd.rearrange("kt p n -> p kt n").bitcast(F32R))
        rows_sb = p_sm.tile([1, 3 * C], F32R, tag="rows")
        nc.scalar.dma_start(rows_sb[:],
                            rowsd.rearrange("r c -> (r c)")[None, :].bitcast(F32R))
        sel_sb = p_sm.tile([H, C], F32R, tag="sel")
        nc.scalar.dma_start(sel_sb[:], c_sel[:].bitcast(F32R))
        bh_sb = p_sm.tile([2, 128], F32R, tag="bh")
        nc.scalar.dma_start(bh_sb[:], c_bh[:].bitcast(F32R))
        b1c = p_sm.tile([128, KH], F32, tag="b1c")
        nc.scalar.dma_start(b1c[:], b1d.rearrange("kh p -> p kh"))
        onesk_h = p_sm.tile([128, 1], FP16, tag="onesk")
        nc.scalar.dma_start(onesk_h[:], c_onesk[:])
        ones_row = p_sm.tile([1, T], F32R, tag="ones")
        nc.scalar.dma_start(ones_row[:], c_ones[:].bitcast(F32R))

        # ---- bulk fp16 weights on the gpsimd queue -----------------------
        def load_fold(tile_ap, dram_t, pat_src, pat_dst, eng, **kw):
            nc_eng = getattr(nc, eng)
            nc_eng.dma_start(tile_ap.rearrange(pat_dst, **kw),
                             dram_t.rearrange(pat_src, p=128, **kw))

        w1_sb = p_big.tile([128, KC * 4 * C], FP16, tag="w1")
        load_fold(w1_sb[:], w1t, "(kt p) m -> p kt m", "p (kt m) -> p kt m",
                  "gpsimd", kt=KC)
        w2_sb = p_big.tile([128, KH * C], FP16, tag="w2")
        load_fold(w2_sb[:], w2t, "(kh p) c -> p kh c", "p (kh c) -> p kh c",
                  "gpsimd", kh=KH)
        wo_sb = p_big.tile([128, KC * C], FP16, tag="wo")
        load_fold(wo_sb[:], wot, "(kt p) c -> p kt c", "p (kt c) -> p kt c",
                  "gpsimd", kt=KC)

        def pccol_r(kt, j, n=1):
            return pc[:, kt * NPC + j:kt * NPC + j + n]

        def pccol(kt, j, n=1):
            return pccol_r(kt, j, n).bitcast(F32)

        eps_col = p_sm.tile([1, 1], F32, tag="epsc")
        nc.vector.memset(eps_col[:], EPS)

        # ---- a path: tt = a/SCAL in [H, T]; clamp; repack to [128, 32] ---
        pa = ps_x.tile([H, T], F32, tag="x", name="pa")
        for kt in range(KC):
            nc.tensor.matmul(pa[:], m_sb[:, kt * H:(kt + 1) * H],
                             xs[:, kt * T:(kt + 1) * T],
                             start=(kt == 0), stop=(kt == KC - 1))
        tt_sb = p_sm.tile([H, T], F32, tag="tts")
        nc.scalar.activation(tt_sb[:], pa[:], AF.Identity,
                             bias=pccol(0, 5)[0:H, :], scale=SCALE / SCAL)
        nc.vector.tensor_scalar_max(tt_sb[:], tt_sb[:], -1.0)
        nc.vector.tensor_scalar_min(tt_sb[:], tt_sb[:], 1.0)
        tt = p_cl.tile([128, FP], F32, tag="tt")
        for b in range(B):
            src = tt_sb[:, b * LC:(b + 1) * LC].rearrange(
                "h (lhi llo) -> h lhi llo", llo=FP)
            nc.sync.dma_start(tt[b * 64:(b + 1) * 64, :], src)

        # ---- both-batch softmax collapse at 64 Chebyshev nodes -----------
        # e chunk broadcast via K=2 matmul; exp + weighted sums per chunk
        pn = p_cl.tile([128, G], F32, tag="ndB")
        zc = p_sm.tile([128, GC], F32, tag="zc")
        nmc = p_sm.tile([128, GC], F32, tag="nmc")
        for i in range(GC):
            sl = slice(i * 512, (i + 1) * 512)
            ep = ps_x.tile([128, 512], F32, tag="x", name=f"ep{i}")
            nc.tensor.matmul(ep[:], bh_sb[:], e2[:, sl], start=True, stop=True)
            nc.scalar.activation(pn[:, sl], ep[:], AF.Exp,
                                 scale=cst_sb[:, O_XN:O_XN + 1].bitcast(F32),
                                 accum_out=zc[:, i:i + 1])
            nc.vector.scalar_tensor_tensor(
                out=pn[:, sl], in0=pn[:, sl], scalar=1.0, in1=ep[:],
                op0=OP.mult, op1=OP.mult, accum_out=nmc[:, i:i + 1])
        z_col = p_sm.tile([128, 1], F32, tag="zcol")
        nc.vector.tensor_reduce(z_col[:], zc[:], axis=AX.X, op=OP.add)
        nm_col = p_sm.tile([128, 1], F32, tag="nmcol")
        nc.vector.tensor_reduce(nm_col[:], nmc[:], axis=AX.X, op=OP.add)
        zr_col = p_sm.tile([128, 1], F32, tag="zrc")
        nc.vector.reciprocal(zr_col[:], z_col[:])
        f_col = p_sm.tile([128, 1], F32, tag="fc")
        nc.vector.tensor_mul(f_col[:], nm_col[:], zr_col[:])
        if DBG:
            nc.sync.dma_start(dbg["d_f"][:], f_col[:])

        # cb[p, k] = ck[batch(p), k] via block-ones matmul (no DRAM trip)
        fdct = p_sm.tile([128, KDEG], F32R, tag="fdct")
        nc.vector.tensor_scalar_mul(
            fdct[:], cst_sb[:, O_DCT:O_DCT + KDEG].bitcast(F32), f_col[:])
        pcb = ps_x.tile([128, KDEG], F32, tag="x", name="pcb")
        nc.tensor.matmul(pcb[:], cst_sb[:, O_BLK:O_BLK + 128], fdct[:],
                         start=True, stop=True)
        cb = p_cl.tile([128, KDEG], F32, tag="cb")
        nc.vector.tensor_copy(cb[:], pcb[:])
        if DBG:
            nc.sync.dma_start(dbg["d_tts"][:], tt_sb[:])
            nc.sync.dma_start(dbg["d_tt"][:], tt[:])
            nc.sync.dma_start(dbg["d_cb"][:], cb[:])

        # ---- Clenshaw over packed a: [128, 32] ---------------------------
        tt2 = p_cl.tile([128, FP], F32, tag="tt2")
        nc.vector.tensor_add(tt2[:], tt[:], tt[:])
        bb1 = p_cl.tile([128, FP], F32, tag="bb1")
        bb2 = p_cl.tile([128, FP], F32, tag="bb2")
        tmp = p_cl.tile([128, FP], F32, tag="tmp")
        nc.vector.memset(bb1[:], 0.0)
        nc.vector.memset(bb2[:], 0.0)
        cur1, cur2 = bb1, bb2
        for k in range(KDEG - 1, 0, -1):
            nc.vector.tensor_mul(tmp[:], tt2[:], cur1[:])
            nc.vector.scalar_tensor_tensor(
                out=cur2[:], in0=tmp[:], scalar=cb[:, k:k + 1], in1=cur2[:],
                op0=OP.add, op1=OP.subtract)
            cur1, cur2 = cur2, cur1
        w_pack = p_cl.tile([128, FP], F32, tag="wp")
        nc.vector.tensor_mul(tmp[:], tt[:], cur1[:])
        nc.vector.scalar_tensor_tensor(
            out=w_pack[:], in0=tmp[:], scalar=cb[:, 0:1], in1=cur2[:],
            op0=OP.add, op1=OP.subtract)
        if DBG:
            nc.sync.dma_start(dbg["d_wp"][:], w_pack[:])

        # ---- w to [H, T] layout (SBUF->SBUF partition repack) ------------
        w_HT = p_sm.tile([H, T], F32R, tag="wht")
        for b in range(B):
            dst = w_HT[:, b * LC:(b + 1) * LC].rearrange(
                "h (lhi llo) -> h lhi llo", llo=FP)
            nc.sync.dma_start(dst, w_pack[b * 64:(b + 1) * 64, :].bitcast(F32R))
        if DBG:
            nc.sync.dma_start(dbg["d_wht"][:], w_HT[:].bitcast(F32))

        # ---- x_attn + residual -> y via selector matmul ------------------
        y_t = []
        for mt in range(KC):
            wr = ps_mm.tile([128, T], F32, tag="mm", name=f"wr{mt}")
            nc.tensor.matmul(wr[:], sel_sb[:, mt * 128:(mt + 1) * 128],
                             w_HT[:], start=True, stop=False)
            nc.tensor.matmul(wr[:], rows_sb[0:1, mt * 128:(mt + 1) * 128],
                             ones_row[:], start=False, stop=True)
            yk = p_act.tile([128, T], FP16, tag="y", name=f"y{mt}")
            # yk = (wr * uv) + xs   (wr already contains w_bcast + cv)
            nc.vector.scalar_tensor_tensor(
                out=yk[:], in0=wr[:], scalar=pccol(mt, 4),
                in1=xs[:, mt * T:(mt + 1) * T].bitcast(F32),
                op0=OP.mult, op1=OP.add)
            y_t.append(yk)
        if DBG:
            for mt in range(KC):
                nc.gpsimd.dma_start(dbg["d_y"][:, mt * T:(mt + 1) * T],
                                    y_t[mt][:])

        g1_row = rows_sb[0:1, C:2 * C]
        g2_row = rows_sb[0:1, 2 * C:3 * C]

        def layernorm(y_tiles, g_row, becol_j, ph):
            stat0 = ps_x.tile([1, T], F32, tag="x", name=f"st0{ph}")
            stat1 = ps_x.tile([1, T], F32, tag="x", name=f"st1{ph}")
            for kt in range(KC):
                nc.tensor.matmul(stat0[:], onesk_h[:], y_tiles[kt][:],
                                 start=(kt == 0), stop=(kt == KC - 1))
            sq_t = []
            for kt in range(KC):
                sq = p_act.tile([128, T], FP16, tag="sq", bufs=2,
                                name=f"sq{ph}{kt}")
                nc.scalar.activation(sq[:], y_tiles[kt][:], AF.Square)
                sq_t.append(sq)
            for kt in range(KC):
                nc.tensor.matmul(stat1[:], onesk_h[:], sq_t[kt][:],
                                 start=(kt == 0), stop=(kt == KC - 1))
            musq_row = p_sm.tile([1, T], F32, tag="lnrow", bufs=4, name=f"musq{ph}")
            nc.scalar.activation(musq_row[:], stat0[:], AF.Square)
            var_row = p_sm.tile([1, T], F32, tag="lnrow", bufs=4, name=f"var{ph}")
            nc.vector.tensor_sub(var_row[:], stat1[:], musq_row[:])
            rstd_f32 = p_sm.tile([1, T], F32, tag="lnrow", bufs=4, name=f"rsf{ph}")
            nc.scalar.activation(rstd_f32[:], var_row[:], AF.Abs_reciprocal_sqrt,
                                 bias=eps_col[:])
            rstd_row = p_sm.tile([1, T], F32R, tag="lnrow", bufs=4, name=f"rstd{ph}")
            nc.vector.tensor_copy(rstd_row[:], rstd_f32[:])
            q_row = p_sm.tile([1, T], F32R, tag="lnrow", bufs=4, name=f"q{ph}")
            nc.vector.tensor_mul(q_row[:], stat0[:], rstd_f32[:])
            outs = []
            for kt in range(KC):
                sl = slice(kt * 128, (kt + 1) * 128)
                pA = ps_mm.tile([128, T], F32, tag="mm", name=f"pA{ph}{kt}")
                nc.tensor.matmul(pA[:], g_row[0:1, sl], rstd_row[:],
                                 start=True, stop=True)
                pB = ps_mm.tile([128, T], F32, tag="mm", name=f"pB{ph}{kt}")
                nc.tensor.matmul(pB[:], g_row[0:1, sl], q_row[:],
                                 start=True, stop=True)
                tx = p_act.tile([128, T], F32, tag="tmpx", bufs=2,
                                name=f"tx{ph}{kt}")
                nc.vector.tensor_mul(tx[:], y_tiles[kt][:], pA[:])
                xo = p_act.tile([128, T], FP16, tag=f"ln{ph}", bufs=4,
                                name=f"ln{ph}{kt}")
                nc.vector.scalar_tensor_tensor(
                    out=xo[:], in0=tx[:], scalar=pccol(kt, becol_j), in1=pB[:],
                    op0=OP.add, op1=OP.subtract)
                outs.append(xo)
            return outs

        x_t = layernorm(y_t, g1_row, 2, "a")
        if DBG:
            for mt in range(KC):
                nc.gpsimd.dma_start(dbg["d_x"][:, mt * T:(mt + 1) * T],
                                    x_t[mt][:])

        # ---- FFN1: h = relu(W1 @ x + b1) ---------------------------------
        h_t = []
        for mt in range(KH):
            pf = ps_mm.tile([128, T], F32, tag="mm", name=f"pf1{mt}")
            for kt in range(KC):
                sl = slice(kt * 4 * C + mt * 128, kt * 4 * C + (mt + 1) * 128)
                nc.tensor.matmul(pf[:], w1_sb[:, sl], x_t[kt][:],
                                 start=(kt == 0), stop=(kt == KC - 1))
            hm = p_big.tile([128, T], FP16, tag="h", bufs=16, name=f"h{mt}")
            nc.scalar.activation(hm[:], pf[:], AF.Relu, bias=b1c[:, mt:mt + 1])
            h_t.append(hm)

        # ---- FFN2 + residual -> y2 ---------------------------------------
        y2_t = []
        for mt in range(KC):
            pf = ps_mm.tile([128, T], F32, tag="mm", name=f"pf2{mt}")
            for kt in range(KH):
                sl = slice(kt * C + mt * 128, kt * C + (mt + 1) * 128)
                nc.tensor.matmul(pf[:], w2_sb[:, sl], h_t[kt][:],
                                 start=(kt == 0), stop=(kt == KH - 1))
            y2 = p_act.tile([128, T], FP16, tag="y", name=f"y2{mt}")
            nc.vector.scalar_tensor_tensor(
                out=y2[:], in0=x_t[mt][:], scalar=pccol(mt, 1),
                in1=pf[:], op0=OP.add, op1=OP.add)
            y2_t.append(y2)

        z_t = layernorm(y2_t, g2_row, 3, "b")

        # ---- output proj: out = Wo @ z + bo ------------------------------
        for mt in range(KC):
            pf = ps_mm.tile([128, T], F32, tag="mm", name=f"pfo{mt}")
            for kt in range(KC):
                sl = slice(kt * C + mt * 128, kt * C + (mt + 1) * 128)
                nc.tensor.matmul(pf[:], wo_sb[:, sl], z_t[kt][:],
                                 start=(kt == 0), stop=(kt == KC - 1))
            om = p_act.tile([128, T], F32, tag="tmpx", bufs=2, name=f"om{mt}")
            nc.scalar.activation(om[:], pf[:], AF.Identity, bias=pccol(mt, 0))
            nc.scalar.dma_start(
                out_sl[:, mt * 128:(mt + 1) * 128, :].rearrange("b c l -> c b l"),
                om[:])

    nc.compile()
    return nc


def kernel(**inputs):
    global _CACHE, LAST_RESULTS
    if _CACHE is None:
        _CACHE = _build()
    nc = _CACHE

    f32 = lambda x: np.asarray(x, dtype=np.float32)
    f16t = lambda x: np.ascontiguousarray(np.asarray(x).T, dtype=np.float16)
    seq = f32(inputs["seq"])

    # host-side stage A: all weight-only precomputation (exact fp32 math)
    Wg = f32(inputs["Wg"])[:, 0]
    bg = f32(inputs["bg"])
    Wk, Wv, Wq = f32(inputs["Wk"]), f32(inputs["Wv"]), f32(inputs["Wq"])
    bq, bv = f32(inputs["bq"]), f32(inputs["bv"])
    uk = Wk @ Wg
    uv = Wv @ Wg
    cv = Wv @ bg + bv
    mask = np.zeros((C, H), np.float32)
    for h in range(H):
        mask[h * D:(h + 1) * D, h] = 1.0
    U = mask * uk[:, None]
    M = (Wq.T @ U).astype(np.float32)                      # [C, H]
    a0s = ((U.T @ bq) * SCALE / SCAL).astype(np.float32)   # [H]

    misc = np.zeros((KC, 128), np.float32)
    misc[0, :H] = a0s
    cols = [f32(inputs["bo"]).reshape(KC, 128), f32(inputs["b2"]).reshape(KC, 128),
            f32(inputs["beta1"]).reshape(KC, 128),
            f32(inputs["beta2"]).reshape(KC, 128),
            uv.reshape(KC, 128), misc]
    pcd = np.ascontiguousarray(np.stack(cols, axis=2))     # [KC, 128, NPC]
    md = np.ascontiguousarray(M.reshape(KC, 128, H))
    b1d = np.ascontiguousarray(f32(inputs["b1"]).reshape(KH, 128))
    rowsd = np.ascontiguousarray(
        np.stack([cv, f32(inputs["g1"]), f32(inputs["g2"])]))

    base = {
        "expv": f32(inputs["exp"]),
        "w1t": f16t(inputs["W1"]),
        "w2t": f16t(inputs["W2"]),
        "wot": f16t(inputs["Wo"]),
        "pcd": pcd,
        "md": md,
        "b1d": b1d,
        "rowsd": rowsd,
    }
    in_maps = []
    for c in range(NCORES):
        m = dict(base)
        m["seq_sl"] = np.ascontiguousarray(seq[:, :, c * LC:(c + 1) * LC])
        in_maps.append(m)

    res = run_bass_kernel_spmd(nc, in_maps, list(range(NCORES)), trace=TRACE,
                               **TRACE_KW)
    LAST_RESULTS = res
    out = np.empty((B, C, L), np.float32)
    for c in range(NCORES):
        out[:, :, c * LC:(c + 1) * LC] = res.results[c]["out_sl"]
    return out


# revision 21
# speedup vs baseline: 1.6070x; 1.0468x over previous
"""Trainium2 Bass kernel for nn_G3DCrossAttention (B=2, C=512, L=2048, G=2048, H=8).

Exact-math rank-1 collapse of the attention (see kernel_v1_baseline.py for the
derivation): exp_p is rank-1 in channels, so per head the attention output is
x_attn = w*u_v + c_v with w = f_b(a), a = x_seq @ M + a0. f_b is evaluated at
64 Chebyshev nodes on device (exact softmax-collapse over all G genes), fit
with a KDEG-term Chebyshev series and evaluated by a Clenshaw recurrence.

v3 structure (vs the 175us baseline):
  - u_k/u_v/c_v/M/a0 depend only on weights -> precomputed host-side in numpy
    and shipped as packed constants (kills the 38us on-device stage A and 3MB
    of Wq/Wk/Wv DMA traffic)
  - e_b node matrix built by a K=2 block-ones matmul from a [2,G] tile instead
    of a broadcast DMA (whose descriptor generation took 21us to issue)
  - Chebyshev coefficients broadcast by a block-ones matmul (no DRAM trip)
  - per-head w broadcast by a selector matmul from a [H,T] tile; c_v folded in
    as a K=1 matmul; the [H,T] tile comes from a 2-DMA SBUF->SBUF repack
  - LN rstd via ACT Abs_reciprocal_sqrt (40000-bucket table; one table switch
    total) instead of single-lane reciprocal (3.3us) or Ln+Exp (table thrash)
  - all constants packed into a handful of DMAs; fp16 weights one DMA each
  - KDEG=16 (w err ~4e-4; full-pipeline fp32 err 2.5e-4; gate is 2e-2)

Sharding: data-parallel over L across 8 cores (L/8 = 256 queries each).
"""

from contextlib import ExitStack

import numpy as np

import concourse.bass as bass
import concourse.tile as tile
from concourse import bacc, mybir
from concourse.bass_utils import run_bass_kernel_spmd

F32 = mybir.dt.float32
F32R = mybir.dt.float32r
FP16 = mybir.dt.float16
AF = mybir.ActivationFunctionType
OP = mybir.AluOpType
AX = mybir.AxisListType

B, C, L, G, H = 2, 512, 2048, 2048, 8
D = C // H
NCORES = 8
LC = L // NCORES              # 256 queries per core
T = B * LC                    # 512 tokens per core (tau = b*LC + l)
KC = C // 128                 # 4 partition tiles over C
KH = (4 * C) // 128           # 16 partition tiles over 4C
FP = LC // 8                  # 32: free dim of the packed a/w tiles
GC = G // 512                 # 4 chunks over genes
SCALE = 1.0 / float(np.sqrt(D))
EPS = 1e-5
SCAL = 5.0                    # Chebyshev half-range in a-units (|a|max ~ 4.43)
KDEG = 16                     # Chebyshev series length
MNODES = 64                   # Chebyshev nodes per batch (2 batches -> 128 parts)
NPC = 6                       # packed per-kt cols: bo b2 be1 be2 uv misc

TRACE = False
TRACE_KW = {}
LAST_RESULTS = None
DBG = False

_CACHE = None


def _consts():
    m = np.arange(MNODES)
    theta = np.pi * (2 * m + 1) / (2 * MNODES)
    xn64 = (SCAL * np.cos(theta)).astype(np.float32)
    xnodes = np.concatenate([xn64, xn64])                 # [128] both batches
    dct1 = np.zeros((MNODES, KDEG), np.float32)
    for k in range(KDEG):
        dct1[:, k] = (2.0 / MNODES) * np.cos(k * theta)
    dct1[:, 0] *= 0.5
    dct_full = np.concatenate([dct1, dct1], axis=0)       # [128, KDEG]
    blockones = np.zeros((128, 128), np.float32)
    blockones[:64, :64] = 1.0
    blockones[64:, 64:] = 1.0
    # cst layout: [dct KDEG][xn 1][blockones 128]
    cst = np.concatenate([dct_full, xnodes[:, None], blockones], axis=1)
    sel = np.zeros((H, C), np.float32)                    # sel[h, c] = [c//D == h]
    for h in range(H):
        sel[h, h * D:(h + 1) * D] = 1.0
    bh = np.zeros((2, 128), np.float32)                   # batch-half selector
    bh[0, :64] = 1.0
    bh[1, 64:] = 1.0
    return cst, sel, bh


def _build():
    nc = bacc.Bacc(debug=False, num_devices=NCORES)

    # ---- external inputs -------------------------------------------------
    seq_sl = nc.dram_tensor("seq_sl", [128, KC * T], F32, kind="ExternalInput")
    expv = nc.dram_tensor("expv", [B, G], F32, kind="ExternalInput")
    w1t = nc.dram_tensor("w1t", [128, KC * 4 * C], FP16, kind="ExternalInput")
    w2t = nc.dram_tensor("w2t", [128, KH * C], FP16, kind="ExternalInput")
    wot = nc.dram_tensor("wot", [128, KC * C], FP16, kind="ExternalInput")
    pcd = nc.dram_tensor("pcd", [128, KC * NPC], F32, kind="ExternalInput")
    md = nc.dram_tensor("md", [128, KC * H], F32, kind="ExternalInput")  # M tiles
    b1d = nc.dram_tensor("b1d", [128, KH], F32, kind="ExternalInput")
    rowsd = nc.dram_tensor("rowsd", [1, 3 * C], F32, kind="ExternalInput")  # cv g1 g2

    out_sl = nc.dram_tensor("out_sl", [B, C, LC], F32, kind="ExternalOutput")

    cst_np, sel_np, bh_np = _consts()
    c_cst = nc.inline_tensor(cst_np, name="c_cst")
    c_sel = nc.inline_tensor(sel_np, name="c_sel")
    c_bh = nc.inline_tensor(bh_np, name="c_bh")
    c_onesk = nc.inline_tensor(np.full((128, 1), 1.0 / C, np.float16),
                               name="c_onesk")
    c_ones = nc.inline_tensor(np.ones((1, B * LC), np.float32), name="c_ones")

    NCST = cst_np.shape[1]
    O_DCT, O_XN, O_BLK = 0, KDEG, KDEG + 1

    dbg = {}
    if DBG:
        for nm, shp in [("d_tts", [H, T]), ("d_tt", [128, FP]),
                        ("d_cb", [128, KDEG]), ("d_wp", [128, FP]),
                        ("d_wht", [H, T]), ("d_y", [128, KC * T]),
                        ("d_x", [128, KC * T]), ("d_f", [128, 1])]:
            dbg[nm] = nc.dram_tensor(nm, shp, F32, kind="ExternalOutput")

    with tile.TileContext(nc) as tc, ExitStack() as ctx:
        p_big = ctx.enter_context(tc.tile_pool(name="big", bufs=1))
        p_act = ctx.enter_context(tc.tile_pool(name="act", bufs=4))
        p_sm = ctx.enter_context(tc.tile_pool(name="sm", bufs=1))
        p_cl = ctx.enter_context(tc.tile_pool(name="cl", bufs=1))
        ps_mm = ctx.enter_context(tc.tile_pool(name="psmm", bufs=4, space="PSUM"))
        ps_x = ctx.enter_context(tc.tile_pool(name="psx", bufs=4, space="PSUM"))

        # ---- critical loads (sync queue): xs then expv -------------------
        xs = p_big.tile([128, KC * T], F32R, tag="xs")
        nc.sync.dma_start(xs[:], seq_sl[:].bitcast(F32R))
        e2 = p_sm.tile([2, G], F32R, tag="e2")
        nc.sync.dma_start(e2[:], expv[:].bitcast(F32R))

        # ---- packed smalls (gpsimd queue; keep ACT queue compute-only) ---
        m_sb = p_sm.tile([128, KC * H], F32R, tag="msb")
        nc.gpsimd.dma_start(m_sb[:], md[:].bitcast(F32R))
        cst_sb = p_sm.tile([128, NCST], F32R, tag="cst")
        nc.gpsimd.dma_start(cst_sb[:], c_cst[:].bitcast(F32R))
        pc = p_sm.tile([128, KC * NPC], F32R, tag="pc")
        nc.gpsimd.dma_start(pc[:], pcd[:].bitcast(F32R))
        rows_sb = p_sm.tile([1, 3 * C], F32R, tag="rows")
        nc.gpsimd.dma_start(rows_sb[:], rowsd[:].bitcast(F32R))
        sel_sb = p_sm.tile([H, C], F32R, tag="sel")
        nc.gpsimd.dma_start(sel_sb[:], c_sel[:].bitcast(F32R))
        bh_sb = p_sm.tile([2, 128], F32R, tag="bh")
        nc.gpsimd.dma_start(bh_sb[:], c_bh[:].bitcast(F32R))
        b1c = p_sm.tile([128, KH], F32, tag="b1c")
        nc.gpsimd.dma_start(b1c[:], b1d[:])
        onesk_h = p_sm.tile([128, 1], FP16, tag="onesk")
        nc.gpsimd.dma_start(onesk_h[:], c_onesk[:])
        ones_row = p_sm.tile([1, T], F32R, tag="ones")
        nc.gpsimd.dma_start(ones_row[:], c_ones[:].bitcast(F32R))

        # ---- bulk fp16 weights on the gpsimd queue -----------------------
        w1_sb = p_big.tile([128, KC * 4 * C], FP16, tag="w1")
        nc.gpsimd.dma_start(w1_sb[:], w1t[:])
        w2_sb = p_big.tile([128, KH * C], FP16, tag="w2")
        nc.gpsimd.dma_start(w2_sb[:], w2t[:])
        wo_sb = p_big.tile([128, KC * C], FP16, tag="wo")
        nc.gpsimd.dma_start(wo_sb[:], wot[:])

        def pccol_r(kt, j, n=1):
            return pc[:, kt * NPC + j:kt * NPC + j + n]

        def pccol(kt, j, n=1):
            return pccol_r(kt, j, n).bitcast(F32)

        eps_col = p_sm.tile([1, 1], F32, tag="epsc")
        nc.vector.memset(eps_col[:], EPS)

        # ---- a path: tt = a/SCAL in [H, T]; clamp; repack to [128, 32] ---
        pa = ps_x.tile([H, T], F32, tag="x", name="pa")
        for kt in range(KC):
            nc.tensor.matmul(pa[:], m_sb[:, kt * H:(kt + 1) * H],
                             xs[:, kt * T:(kt + 1) * T],
                             start=(kt == 0), stop=(kt == KC - 1))
        tt_sb = p_sm.tile([H, T], F32, tag="tts")
        nc.scalar.activation(tt_sb[:], pa[:], AF.Identity,
                             bias=pccol(0, 5)[0:H, :], scale=SCALE / SCAL)
        nc.vector.tensor_scalar_max(tt_sb[:], tt_sb[:], -1.0)
        nc.vector.tensor_scalar_min(tt_sb[:], tt_sb[:], 1.0)
        tt = p_cl.tile([128, FP], F32, tag="tt")
        for b in range(B):
            src = tt_sb[:, b * LC:(b + 1) * LC].rearrange(
                "h (lhi llo) -> h lhi llo", llo=FP)
            nc.sync.dma_start(tt[b * 64:(b + 1) * 64, :], src)

        # ---- both-batch softmax collapse at 64 Chebyshev nodes -----------
        # e chunk broadcast via K=2 matmul; exp + weighted sums per chunk
        pn = p_cl.tile([128, G], F32, tag="ndB")
        zc = p_sm.tile([128, GC], F32, tag="zc")
        nmc = p_sm.tile([128, GC], F32, tag="nmc")
        for i in range(GC):
            sl = slice(i * 512, (i + 1) * 512)
            ep = ps_x.tile([128, 512], F32, tag="x", name=f"ep{i}")
            nc.tensor.matmul(ep[:], bh_sb[:], e2[:, sl], start=True, stop=True)
            nc.scalar.activation(pn[:, sl], ep[:], AF.Exp,
                                 scale=cst_sb[:, O_XN:O_XN + 1].bitcast(F32),
                                 accum_out=zc[:, i:i + 1])
            nc.vector.scalar_tensor_tensor(
                out=pn[:, sl], in0=pn[:, sl], scalar=1.0, in1=ep[:],
                op0=OP.mult, op1=OP.mult, accum_out=nmc[:, i:i + 1])
        z_col = p_sm.tile([128, 1], F32, tag="zcol")
        nc.vector.tensor_reduce(z_col[:], zc[:], axis=AX.X, op=OP.add)
        nm_col = p_sm.tile([128, 1], F32, tag="nmcol")
        nc.vector.tensor_reduce(nm_col[:], nmc[:], axis=AX.X, op=OP.add)
        zr_col = p_sm.tile([128, 1], F32, tag="zrc")
        nc.vector.reciprocal(zr_col[:], z_col[:])
        f_col = p_sm.tile([128, 1], F32, tag="fc")
        nc.vector.tensor_mul(f_col[:], nm_col[:], zr_col[:])
        if DBG:
            nc.sync.dma_start(dbg["d_f"][:], f_col[:])

        # cb[p, k] = ck[batch(p), k] via block-ones matmul (no DRAM trip)
        fdct = p_sm.tile([128, KDEG], F32R, tag="fdct")
        nc.vector.tensor_scalar_mul(
            fdct[:], cst_sb[:, O_DCT:O_DCT + KDEG].bitcast(F32), f_col[:])
        pcb = ps_x.tile([128, KDEG], F32, tag="x", name="pcb")
        nc.tensor.matmul(pcb[:], cst_sb[:, O_BLK:O_BLK + 128], fdct[:],
                         start=True, stop=True)
        cb = p_cl.tile([128, KDEG], F32, tag="cb")
        nc.vector.tensor_copy(cb[:], pcb[:])
        if DBG:
            nc.sync.dma_start(dbg["d_tts"][:], tt_sb[:])
            nc.sync.dma_start(dbg["d_tt"][:], tt[:])
            nc.sync.dma_start(dbg["d_cb"][:], cb[:])

        # ---- Clenshaw over packed a: [128, 32] ---------------------------
        tt2 = p_cl.tile([128, FP], F32, tag="tt2")
        nc.vector.tensor_add(tt2[:], tt[:], tt[:])
        bb1 = p_cl.tile([128, FP], F32, tag="bb1")
        bb2 = p_cl.tile([128, FP], F32, tag="bb2")
        tmp = p_cl.tile([128, FP], F32, tag="tmp")
        nc.vector.memset(bb1[:], 0.0)
        nc.vector.memset(bb2[:], 0.0)
        cur1, cur2 = bb1, bb2
        for k in range(KDEG - 1, 0, -1):
            nc.vector.tensor_mul(tmp[:], tt2[:], cur1[:])
            nc.vector.scalar_tensor_tensor(
                out=cur2[:], in0=tmp[:], scalar=cb[:, k:k + 1], in1=cur2[:],
                op0=OP.add, op1=OP.subtract)
            cur1, cur2 = cur2, cur1
        w_pack = p_cl.tile([128, FP], F32, tag="wp")
        nc.vector.tensor_mul(tmp[:], tt[:], cur1[:])
        nc.vector.scalar_tensor_tensor(
            out=w_pack[:], in0=tmp[:], scalar=cb[:, 0:1], in1=cur2[:],
            op0=OP.add, op1=OP.subtract)
        if DBG:
            nc.sync.dma_start(dbg["d_wp"][:], w_pack[:])

        # ---- w to [H, T] layout (SBUF->SBUF partition repack) ------------
        w_HT = p_sm.tile([H, T], F32R, tag="wht")
        for b in range(B):
            dst = w_HT[:, b * LC:(b + 1) * LC].rearrange(
                "h (lhi llo) -> h lhi llo", llo=FP)
            nc.sync.dma_start(dst, w_pack[b * 64:(b + 1) * 64, :].bitcast(F32R))
        if DBG:
            nc.sync.dma_start(dbg["d_wht"][:], w_HT[:].bitcast(F32))

        # ---- x_attn + residual -> y via selector matmul ------------------
        y_t = []
        for mt in range(KC):
            wr = ps_mm.tile([128, T], F32, tag="mm", name=f"wr{mt}")
            nc.tensor.matmul(wr[:], sel_sb[:, mt * 128:(mt + 1) * 128],
                             w_HT[:], start=True, stop=False)
            nc.tensor.matmul(wr[:], rows_sb[0:1, mt * 128:(mt + 1) * 128],
                             ones_row[:], start=False, stop=True)
            yk = p_act.tile([128, T], FP16, tag="y", name=f"y{mt}")
            # yk = (wr * uv) + xs   (wr already contains w_bcast + cv)
            nc.vector.scalar_tensor_tensor(
                out=yk[:], in0=wr[:], scalar=pccol(mt, 4),
                in1=xs[:, mt * T:(mt + 1) * T].bitcast(F32),
                op0=OP.mult, op1=OP.add)
            y_t.append(yk)
        if DBG:
            for mt in range(KC):
                nc.gpsimd.dma_start(dbg["d_y"][:, mt * T:(mt + 1) * T],
                                    y_t[mt][:])

        g1_row = rows_sb[0:1, C:2 * C]
        g2_row = rows_sb[0:1, 2 * C:3 * C]

        def layernorm(y_tiles, g_row, becol_j, ph):
            stat0 = ps_x.tile([1, T], F32, tag="x", name=f"st0{ph}")
            stat1 = ps_x.tile([1, T], F32, tag="x", name=f"st1{ph}")
            for kt in range(KC):
                nc.tensor.matmul(stat0[:], onesk_h[:], y_tiles[kt][:],
                                 start=(kt == 0), stop=(kt == KC - 1))
            sq_t = []
            for kt in range(KC):
                sq = p_act.tile([128, T], FP16, tag="sq", bufs=2,
                                name=f"sq{ph}{kt}")
                nc.scalar.activation(sq[:], y_tiles[kt][:], AF.Square)
                sq_t.append(sq)
            for kt in range(KC):
                nc.tensor.matmul(stat1[:], onesk_h[:], sq_t[kt][:],
                                 start=(kt == 0), stop=(kt == KC - 1))
            musq_row = p_sm.tile([1, T], F32, tag="lnrow", bufs=4, name=f"musq{ph}")
            nc.scalar.activation(musq_row[:], stat0[:], AF.Square)
            var_row = p_sm.tile([1, T], F32, tag="lnrow", bufs=4, name=f"var{ph}")
            nc.vector.tensor_sub(var_row[:], stat1[:], musq_row[:])
            rstd_f32 = p_sm.tile([1, T], F32, tag="lnrow", bufs=4, name=f"rsf{ph}")
            nc.scalar.activation(rstd_f32[:], var_row[:], AF.Abs_reciprocal_sqrt,
                                 bias=eps_col[:])
            rstd_row = p_sm.tile([1, T], F32R, tag="lnrow", bufs=4, name=f"rstd{ph}")
            nc.vector.tensor_copy(rstd_row[:], rstd_f32[:])
            q_row = p_sm.tile([1, T], F32R, tag="lnrow", bufs=4, name=f"q{ph}")
            nc.vector.tensor_mul(q_row[:], stat0[:], rstd_f32[:])
            outs = []
            for kt in range(KC):
                sl = slice(kt * 128, (kt + 1) * 128)
                pA = ps_mm.tile([128, T], F32, tag="mm", name=f"pA{ph}{kt}")
                nc.tensor.matmul(pA[:], g_row[0:1, sl], rstd_row[:],
                                 start=True, stop=True)
                pB = ps_mm.tile([128, T], F32, tag="mm", name=f"pB{ph}{kt}")
                nc.tensor.matmul(pB[:], g_row[0:1, sl], q_row[:],
                                 start=True, stop=True)
                tx = p_act.tile([128, T], F32, tag="tmpx", bufs=2,
                                name=f"tx{ph}{kt}")
                nc.vector.tensor_mul(tx[:], y_tiles[kt][:], pA[:])
                xo = p_act.tile([128, T], FP16, tag=f"ln{ph}", bufs=4,
                                name=f"ln{ph}{kt}")
                nc.vector.scalar_tensor_tensor(
                    out=xo[:], in0=tx[:], scalar=pccol(kt, becol_j), in1=pB[:],
                    op0=OP.add, op1=OP.subtract)
                outs.append(xo)
            return outs

        x_t = layernorm(y_t, g1_row, 2, "a")
        if DBG:
            for mt in range(KC):
                nc.gpsimd.dma_start(dbg["d_x"][:, mt * T:(mt + 1) * T],
                                    x_t[mt][:])

        # ---- FFN1: h = relu(W1 @ x + b1) ---------------------------------
        h_t = []
        for mt in range(KH):
            pf = ps_mm.tile([128, T], F32, tag="mm", name=f"pf1{mt}")
            for kt in range(KC):
                sl = slice(kt * 4 * C + mt * 128, kt * 4 * C + (mt + 1) * 128)
                nc.tensor.matmul(pf[:], w1_sb[:, sl], x_t[kt][:],
                                 start=(kt == 0), stop=(kt == KC - 1))
            hm = p_big.tile([128, T], FP16, tag="h", bufs=16, name=f"h{mt}")
            nc.scalar.activation(hm[:], pf[:], AF.Relu, bias=b1c[:, mt:mt + 1])
            h_t.append(hm)

        # ---- FFN2 + residual -> y2 ---------------------------------------
        y2_t = []
        for mt in range(KC):
            pf = ps_mm.tile([128, T], F32, tag="mm", name=f"pf2{mt}")
            for kt in range(KH):
                sl = slice(kt * C + mt * 128, kt * C + (mt + 1) * 128)
                nc.tensor.matmul(pf[:], w2_sb[:, sl], h_t[kt][:],
                                 start=(kt == 0), stop=(kt == KH - 1))
            y2 = p_act.tile([128, T], FP16, tag="y", name=f"y2{mt}")
            nc.vector.scalar_tensor_tensor(
                out=y2[:], in0=x_t[mt][:], scalar=pccol(mt, 1),
                in1=pf[:], op0=OP.add, op1=OP.add)
            y2_t.append(y2)

        z_t = layernorm(y2_t, g2_row, 3, "b")

        # ---- output proj: out = Wo @ z + bo ------------------------------
        for mt in range(KC):
            pf = ps_mm.tile([128, T], F32, tag="mm", name=f"pfo{mt}")
            for kt in range(KC):
                sl = slice(kt * C + mt * 128, kt * C + (mt + 1) * 128)
                nc.tensor.matmul(pf[:], wo_sb[:, sl], z_t[kt][:],
                                 start=(kt == 0), stop=(kt == KC - 1))
            om = p_act.tile([128, T], F32, tag="tmpx", bufs=2, name=f"om{mt}")
            nc.scalar.activation(om[:], pf[:], AF.Identity, bias=pccol(mt, 0))
            nc.sync.dma_start(
                out_sl[:, mt * 128:(mt + 1) * 128, :].rearrange("b c l -> c b l"),
                om[:])

    nc.compile()
    return nc


def kernel(**inputs):
    global _CACHE, LAST_RESULTS
    if _CACHE is None:
        _CACHE = _build()
    nc = _CACHE

    f32 = lambda x: np.asarray(x, dtype=np.float32)
    f16t = lambda x: np.ascontiguousarray(np.asarray(x).T, dtype=np.float16)
    seq = f32(inputs["seq"])

    # host-side stage A: all weight-only precomputation (exact fp32 math)
    Wg = f32(inputs["Wg"])[:, 0]
    bg = f32(inputs["bg"])
    Wk, Wv, Wq = f32(inputs["Wk"]), f32(inputs["Wv"]), f32(inputs["Wq"])
    bq, bv = f32(inputs["bq"]), f32(inputs["bv"])
    uk = Wk @ Wg
    uv = Wv @ Wg
    cv = Wv @ bg + bv
    mask = np.zeros((C, H), np.float32)
    for h in range(H):
        mask[h * D:(h + 1) * D, h] = 1.0
    U = mask * uk[:, None]
    M = (Wq.T @ U).astype(np.float32)                      # [C, H]
    a0s = ((U.T @ bq) * SCALE / SCAL).astype(np.float32)   # [H]

    misc = np.zeros((KC, 128), np.float32)
    misc[0, :H] = a0s
    cols = [f32(inputs["bo"]).reshape(KC, 128), f32(inputs["b2"]).reshape(KC, 128),
            f32(inputs["beta1"]).reshape(KC, 128),
            f32(inputs["beta2"]).reshape(KC, 128),
            uv.reshape(KC, 128), misc]
    # device-image layouts: [128, X] exactly as the SBUF tile will hold them
    pcd = np.ascontiguousarray(
        np.stack(cols, axis=2).transpose(1, 0, 2).reshape(128, KC * NPC))
    md = np.ascontiguousarray(
        M.reshape(KC, 128, H).transpose(1, 0, 2).reshape(128, KC * H))
    b1d = np.ascontiguousarray(f32(inputs["b1"]).reshape(KH, 128).T)
    rowsd = np.ascontiguousarray(
        np.stack([cv, f32(inputs["g1"]), f32(inputs["g2"])]).reshape(1, 3 * C))

    def wimg(wT_f16, ksplit):
        # [Cin, Cout] -> [128, ksplit*Cout] image (partition-folded)
        cin, cout = wT_f16.shape
        return np.ascontiguousarray(
            wT_f16.reshape(ksplit, 128, cout).transpose(1, 0, 2).reshape(
                128, ksplit * cout))

    base = {
        "expv": f32(inputs["exp"]),
        "w1t": wimg(f16t(inputs["W1"]), KC),
        "w2t": wimg(f16t(inputs["W2"]), KH),
        "wot": wimg(f16t(inputs["Wo"]), KC),
        "pcd": pcd,
        "md": md,
        "b1d": b1d,
        "rowsd": rowsd,
    }
    # xs image per core: [128, kt*T + b*LC + l] = seq[b, kt*128+p, c0+l]
    seq_r = seq.reshape(B, KC, 128, L)
    in_maps = []
    for c in range(NCORES):
        m = dict(base)
        sl = seq_r[:, :, :, c * LC:(c + 1) * LC]           # [B, KC, 128, LC]
        m["seq_sl"] = np.ascontiguousarray(
            sl.transpose(2, 1, 0, 3).reshape(128, KC * T))
        in_maps.append(m)

    res = run_bass_kernel_spmd(nc, in_maps, list(range(NCORES)), trace=TRACE,
                               **TRACE_KW)
    LAST_RESULTS = res
    out = np.empty((B, C, L), np.float32)
    for c in range(NCORES):
        out[:, :, c * LC:(c + 1) * LC] = res.results[c]["out_sl"]
    return out
